# revision 29
# baseline (speedup 1.0000x reference)
"""Multi-head attention (B=2, S=2048, D=1024, H=16 heads, causal) on 8 trn2 cores.

Sharding: heads across cores (2 heads = 128 channels per core).
  - W_q/W_k/W_v column-sharded: each core projects all tokens to its 128 channels.
  - Attention per (batch, head) fully local to a core.
  - W_o row-sharded: each core computes a partial output projection; partials
    are summed on the host (the unshard step), then b_o (+ W_o @ b_v) is added.

Device layout: Q/K transposed (channels on partitions, tokens on free).
  - Scores computed as S^T blocks [128 k-tok, 512 q-tok] so exp is elementwise.
  - V^T produced directly by the projection (lhsT=x chunk, rhs=W_v chunk), no
    PE transposes. Each (b, head) V block carries 64 all-ones columns, so the
    AV matmul emits the softmax sums pre-broadcast across 64 partitions; the
    normalize is then a single tensor-tensor divide per head.
  - Causal structure: scores/exp/AV restricted to the valid q-range per
    k-block; the 127-wide diagonal triangle is multiplied in on GpSimd.
  - Projections / V^T pieces / output-projection pieces are emitted as filler
    between attention blocks so the PE never idles (keeps the 2.4 GHz p-state)
    while the scalar engine runs the exp stream.

All matmuls run in bf16 (inputs cast on host) with fp32 PSUM accumulation;
the partial output is returned bf16 and reduced in fp32 on the host.
"""

import sys
from collections import deque
from functools import partial

import numpy as np

try:
    import concourse.bass as bass  # noqa: F401
except ImportError:  # pragma: no cover
    sys.path.insert(0, "/opt/trn_rl_repo")

import ml_dtypes

import concourse.mybir as mybir
import concourse.tile as tile
from concourse import bacc, bass_utils
from concourse.masks import make_identity

P = 128
B, S, D = 2, 2048, 1024
H, DK = 16, 64
N_CORES = 8
HPC = H // N_CORES  # heads per core = 2
CH = HPC * DK  # channels per core = 128
TOK = B * S  # 4096
NKB = S // P  # k-blocks per batch = 16
CW = 512  # q column width
NJ = S // CW  # q columns per batch = 4
NTG = S // CW  # 512-token projection groups per batch = 4
KPG = CW // P  # k-blocks per token group = 4
XC = D // P  # x-dim chunks = 8
MO = D // P  # output-channel chunks = 8

BF16 = mybir.dt.bfloat16
F32 = mybir.dt.float32
NPBF16 = ml_dtypes.bfloat16

_BUILD_CACHE = {}


def _analyze_mask(mask):
    """Block plan from the (1,1,S,S) boolean mask (shared across batch/head).

    plan[j] = tuple of (bk, qa, mixed) for each k-block with any valid entry:
      qa    = first local q with any valid k; scores/exp/AV cover [qa, CW).
      mixed = None or (pat_off, a, w): a2[:, :, a:a+w] *= pattern columns.
    Patterns are deduplicated and concatenated into pats (P, W_total) in
    [k, q] layout.
    """
    m = np.asarray(mask).reshape(S, S).astype(bool)  # m[q, k]
    pat_index = {}
    pat_list = []
    plan = []
    for j in range(NJ):
        q0 = j * CW
        blocks = []
        first = True
        for bk in range(NKB):
            sub = m[q0 : q0 + CW, bk * P : (bk + 1) * P]  # (CW q, P k)
            anyv = sub.any(axis=1)
            if not anyv.any():
                continue
            qa = int(np.argmax(anyv))
            if first:
                # the first block initializes the whole PSUM accumulator
                qa = 0
                first = False
            validall = sub.all(axis=1)
            nfv = ~validall
            nfv[:qa] = False
            mixed = None
            if nfv.any():
                idx = np.where(nfv)[0]
                a_, b_ = int(idx[0]), int(idx[-1]) + 1
                patt = np.ascontiguousarray(sub[a_:b_, :].T).astype(np.float32)
                key = (patt.shape[1], patt.tobytes())
                if key not in pat_index:
                    pat_index[key] = len(pat_list)
                    pat_list.append(patt)
                mixed = (pat_index[key], a_, b_ - a_)
            blocks.append((bk, qa, mixed))
        plan.append(tuple(blocks))
    offs = [0]
    for p_ in pat_list:
        offs.append(offs[-1] + p_.shape[1])
    plan2 = []
    for col in plan:
        col2 = []
        for bk, qa, mixed in col:
            if mixed is not None:
                pid, a_, w_ = mixed
                mixed = (offs[pid], a_, w_)
            col2.append((bk, qa, mixed))
        plan2.append(tuple(col2))
    if pat_list:
        pat_arr = np.concatenate(pat_list, axis=1)  # (P, W_total)
    else:
        pat_arr = np.ones((P, 1), np.float32)
    return tuple(plan2), pat_arr


def _build(plan, pat_w):
    nc = bacc.Bacc(
        "TRN2",
        target_bir_lowering=False,
        debug=False,
        enable_asserts=True,
        num_devices=N_CORES,
    )
    NTT = B * NTG
    xq = nc.dram_tensor("xq", [NTT, P, XC, CW], BF16, kind="ExternalInput").ap()
    xk = nc.dram_tensor("xk", [NTT, P, XC, CW], BF16, kind="ExternalInput").ap()
    xv = nc.dram_tensor("xv", [NTT, P, XC, CW], BF16, kind="ExternalInput").ap()
    wq = nc.dram_tensor("wq", [D, CH], BF16, kind="ExternalInput").ap()
    wk = nc.dram_tensor("wk", [D, CH], BF16, kind="ExternalInput").ap()
    wv = nc.dram_tensor("wv", [D, CH], BF16, kind="ExternalInput").ap()
    wo = nc.dram_tensor("wo", [CH, D], BF16, kind="ExternalInput").ap()
    bq = nc.dram_tensor("bq", [CH, 1], F32, kind="ExternalInput").ap()
    bk_ = nc.dram_tensor("bk", [CH, 1], F32, kind="ExternalInput").ap()
    mpat = nc.dram_tensor("mpat", [P, pat_w], BF16, kind="ExternalInput").ap()
    out = nc.dram_tensor(
        "out", [MO, B * NJ, P, CW], BF16, kind="ExternalOutput"
    ).ap()

    xdram = {"q": xq, "k": xk, "v": xv}

    with tile.TileContext(nc) as tc:
        with (
            tc.tile_pool(name="const", bufs=1) as const,
            tc.tile_pool(name="persist", bufs=1) as persist,
            tc.tile_pool(name="xt", bufs=2) as xtp,
            tc.tile_pool(name="vt", bufs=2) as vtp,
            tc.tile_pool(name="a2", bufs=4) as a2p,
            tc.tile_pool(name="nrm", bufs=2) as nrm,
            tc.tile_pool(name="yt", bufs=3) as ytp,
            tc.tile_pool(name="ob", bufs=10) as obp,
            tc.tile_pool(name="pp", bufs=2, space="PSUM") as pp,
            tc.tile_pool(name="s2", bufs=2, space="PSUM") as s2p,
            tc.tile_pool(name="op", bufs=2, space="PSUM") as opsp,
        ):
            # --- constants: loaded via the Activation HWDGE queue so the SP
            # queue is dedicated to x prefetch ------------------------------
            w_sb = {}
            b_sb = {}
            for name, wdram, bdram in (
                ("q", wq, bq),
                ("k", wk, bk_),
                ("v", wv, None),
            ):
                w_sb[name] = const.tile(
                    [P, XC, CH], BF16, tag=f"w{name}", name=f"w{name}"
                )
                wview = wdram.rearrange("(o p) c -> p o c", p=P)
                for h in range(0, XC, 2):
                    nc.scalar.dma_start(
                        w_sb[name][:, h : h + 2, :], wview[:, h : h + 2, :]
                    )
                if bdram is not None:
                    b_sb[name] = const.tile(
                        [CH, 1], F32, tag=f"b{name}", name=f"b{name}"
                    )
                    nc.scalar.dma_start(b_sb[name][:], bdram)
            wo_sb = const.tile([CH, D], BF16, tag="wo")
            for h in range(0, D, CW):
                nc.scalar.dma_start(wo_sb[:, h : h + CW], wo[:, h : h + CW])
            mask_sb = const.tile([P, pat_w], BF16, tag="mpat")
            nc.scalar.dma_start(mask_sb[:], mpat)
            ident = const.tile([P, P], BF16, tag="ident")
            make_identity(nc, ident)

            # persistent Q^T/K^T [chan, tok] and V^T-augmented per batch
            qt, kt = {}, {}
            for b in range(B):
                qt[b] = persist.tile([CH, S], BF16, tag=f"qt{b}", name=f"qt{b}")
                kt[b] = persist.tile([CH, S], BF16, tag=f"kt{b}", name=f"kt{b}")
            # vaug[b]: [k-tok, NKB, head, 64 V cols + 64 ones cols]; the ones
            # make the AV matmul emit softmax sums broadcast on rows 64..127
            vaug = {}
            for b in range(B):
                vaug[b] = persist.tile(
                    [P, NKB, HPC, P], BF16, tag=f"vaug{b}", name=f"vaug{b}"
                )
                # contiguous fill; the V columns are overwritten by the V^T
                # evacs, leaving the ones columns that produce the sums rows
                nc.gpsimd.memset(vaug[b][:], 1.0)

            def load_x(b, tg, fine=False):
                """Prefetch x tiles; fine=True splits into 128KB chunks for a
                fast first matmul, otherwise 256KB halves the SP setup cost."""
                tiles = {}
                g = b * NTG + tg
                step = 1 if fine else 2
                for name in ("q", "k", "v"):
                    t = xtp.tile([P, XC, CW], BF16, tag=f"x{name}")
                    if fine and name == "q":
                        # halve the very first chunk so matmul 0 starts sooner
                        hw = CW // 2
                        nc.sync.dma_start(
                            t[:, 0, 0:hw], xdram[name][g, :, 0, 0:hw]
                        )
                        nc.sync.dma_start(
                            t[:, 0, hw:], xdram[name][g, :, 0, hw:]
                        )
                        first = 1
                    else:
                        first = 0
                    for xc in range(first, XC, step):
                        nc.sync.dma_start(
                            t[:, xc : xc + step, :],
                            xdram[name][g, :, xc : xc + step, :],
                        )
                    tiles[name] = t
                return tiles

            def emit_proj(b, name, tg, xtile):
                """Q/K projection of one 512-token group -> qt/kt columns."""
                ps = pp.tile([CH, CW], F32, tag="pp")
                for xc in range(XC):
                    nc.tensor.matmul(
                        ps[:],
                        lhsT=w_sb[name][:, xc, :],
                        rhs=xtile[:, xc, :],
                        start=(xc == 0),
                        stop=(xc == XC - 1),
                    )
                dst = qt if name == "q" else kt
                nc.vector.tensor_add(
                    dst[b][:, tg * CW : (tg + 1) * CW],
                    ps[:],
                    b_sb[name][:, 0:1].to_broadcast((CH, CW)),
                )

            def emit_proj_v(b, tg, xtile, vt_sb):
                """V projection of one token group in [chan, tok] layout."""
                ps = pp.tile([CH, CW], F32, tag="pp")
                for xc in range(XC):
                    nc.tensor.matmul(
                        ps[:],
                        lhsT=w_sb["v"][:, xc, :],
                        rhs=xtile[:, xc, :],
                        start=(xc == 0),
                        stop=(xc == XC - 1),
                    )
                nc.vector.tensor_copy(vt_sb[:], ps[:])

            def vt_unit(b, tg, tb, vt_sb):
                """Transpose one 128-token block of V into vaug [tok, chan]."""
                tp_ = pp.tile([P, HPC, DK], BF16, tag="pp")
                nc.tensor.transpose(
                    tp_[:], vt_sb[:, tb * P : (tb + 1) * P], ident[:]
                )
                kb = tg * KPG + tb
                nc.vector.tensor_copy(vaug[b][:, kb, :, 0:DK], tp_[:])

            def oproj_piece(tcol, yt, mo, last=False):
                op_ps = pp.tile([P, CW], F32, tag="pp")
                nc.tensor.matmul(
                    op_ps[:],
                    lhsT=wo_sb[:, mo * P : (mo + 1) * P],
                    rhs=yt[:],
                    start=True,
                    stop=True,
                )
                ob = obp.tile([P, CW], BF16, tag="ob")
                # evac split 2:6 ACT:DVE (the last column alternates evenly
                # and splits DMAs to shorten the kernel tail)
                on_act = mo % 2 == 0 if last else mo % 4 == 0
                if on_act:
                    nc.scalar.copy(ob[:], op_ps[:])
                else:
                    nc.vector.tensor_copy(ob[:], op_ps[:])
                if last:
                    # split across the SP and Pool DGE queues for a short tail
                    h = CW // 2
                    nc.sync.dma_start(out[mo, tcol][:, 0:h], ob[:, 0:h])
                    nc.gpsimd.dma_start(out[mo, tcol][:, h:], ob[:, h:])
                else:
                    nc.sync.dma_start(out[mo, tcol], ob[:])

            def attention_col(b, j, tg, vt_units, nxt_units, lazy, pace=450):
                """One 512-wide q column; drains filler units between blocks."""
                blocks = plan[j]
                q0 = j * CW
                yt = ytp.tile([CH, CW], BF16, tag="yt")
                if not blocks:
                    while vt_units:
                        _, fn = vt_units.popleft()
                        fn()
                    nc.gpsimd.memset(yt[:], 0.0)
                    return yt
                ops = [
                    opsp.tile([P, CW], F32, tag="op", name=f"op{hl}")
                    for hl in range(HPC)
                ]
                nblk = len(blocks)

                def emit_av(i, bk, qa, a2):
                    for hl in range(HPC):
                        nc.tensor.matmul(
                            ops[hl][:, qa:],
                            lhsT=vaug[b][:, bk, hl, :],
                            rhs=a2[:, hl, qa:],
                            start=(i == 0),
                            stop=(i == nblk - 1),
                            skip_group_check=True,
                        )

                pend = None
                debt = 0
                for i, (bk, qa, mixed) in enumerate(blocks):
                    k0 = bk * P
                    s2 = s2p.tile([P, HPC, CW], F32, tag="s2")
                    for hl in range(HPC):
                        hs = slice(hl * DK, (hl + 1) * DK)
                        nc.tensor.matmul(
                            s2[:, hl, qa:],
                            lhsT=kt[b][hs, k0 : k0 + P],
                            rhs=qt[b][hs, q0 + qa : q0 + CW],
                            start=True,
                            stop=True,
                            skip_group_check=True,
                        )
                    a2 = a2p.tile([P, HPC, CW], BF16, tag="a2")
                    nc.scalar.activation(
                        a2[:, :, qa:],
                        s2[:, :, qa:],
                        mybir.ActivationFunctionType.Exp,
                        scale=0.125,
                    )
                    if mixed is not None:
                        off, a_, w_ = mixed
                        nc.gpsimd.tensor_tensor(
                            a2[:, :, a_ : a_ + w_],
                            a2[:, :, a_ : a_ + w_],
                            mask_sb[:, None, off : off + w_].to_broadcast(
                                (P, HPC, w_)
                            ),
                            mybir.AluOpType.mult,
                        )
                    if pend is not None:
                        # this tg's V^T blocks must land before its AVs
                        if pend[1] >= tg * KPG:
                            while vt_units:
                                _, fn = vt_units.popleft()
                                fn()
                        emit_av(*pend)
                    pend = (i, bk, qa, a2)
                    # filler: keep the PE fed while the scalar engine exps
                    debt += pace
                    while debt > 0 and (vt_units or nxt_units or lazy):
                        src = (
                            vt_units
                            if vt_units
                            else (nxt_units if nxt_units else lazy)
                        )
                        ns, fn = src.popleft()
                        fn()
                        debt -= ns
                if pend[1] >= tg * KPG:
                    while vt_units:
                        _, fn = vt_units.popleft()
                        fn()
                emit_av(*pend)
                # normalize: rows 64..127 of ops are the sums, pre-broadcast.
                # DVE reads at most one PSUM operand per op, so reciprocal the
                # sums into SBUF, then multiply against the PSUM values.
                for hl in range(HPC):
                    # reciprocal_approx_fast misreads PSUM on HW — bounce the
                    # sums through SBUF (on ACT, to shorten the DVE chain)
                    sums = nrm.tile([DK, CW], F32, tag="sums", name=f"sums{hl}")
                    nc.scalar.copy(sums[:], ops[hl][DK:P, :])
                    rec = nrm.tile([DK, CW], F32, tag="rec", name=f"rec{hl}")
                    nc.vector.reciprocal_approx_fast(out=rec[:], in_=sums[:])
                    nc.vector.tensor_tensor(
                        yt[hl * DK : (hl + 1) * DK, :],
                        ops[hl][0:DK, :],
                        rec[:],
                        mybir.AluOpType.mult,
                    )
                return yt

            # --- main schedule ---------------------------------------------
            seq = [(b, tg) for tg in range(NTG) for b in range(B)]
            cur_x = load_x(*seq[0], fine=True)
            lazy = deque()
            for si, (b, tg) in enumerate(seq):
                nxt_x = load_x(*seq[si + 1]) if si + 1 < len(seq) else None
                emit_proj(b, "q", tg, cur_x["q"])
                emit_proj(b, "k", tg, cur_x["k"])
                vt_sb = vtp.tile([CH, CW], BF16, tag="vt")
                emit_proj_v(b, tg, cur_x["v"], vt_sb)
                vt_units = deque(
                    (450, partial(vt_unit, b, tg, tb, vt_sb))
                    for tb in range(KPG)
                )
                last = si == len(seq) - 1
                yt = attention_col(
                    b, tg, tg, vt_units, deque(), lazy,
                    pace=150 if last else 450,
                )
                while vt_units:
                    _, fn = vt_units.popleft()
                    fn()
                tcol = b * NJ + tg
                lazy.extend(
                    (450, partial(oproj_piece, tcol, yt, mo, last))
                    for mo in range(MO)
                )
                cur_x = nxt_x
            while lazy:
                _, fn = lazy.popleft()
                fn()
    nc.compile()
    return nc


def _get_module(plan, pat_w):
    key = (plan, pat_w)
    if key not in _BUILD_CACHE:
        _BUILD_CACHE[key] = _build(plan, pat_w)
    return _BUILD_CACHE[key]


def _prep_inputs(query, key, value, mask, W_q, b_q, W_k, b_k, W_v, b_v, W_o, b_o):
    def xt_of(x):
        x2 = np.asarray(x, np.float32).reshape(TOK, D)
        xt = x2.T.astype(NPBF16)  # (D, TOK)
        xt = xt.reshape(XC, P, B * NTG, CW).transpose(2, 1, 0, 3)
        return np.ascontiguousarray(xt)  # (NTT, P, XC, CW)

    xq, xk, xv = xt_of(query), xt_of(key), xt_of(value)
    plan, pat_arr = _analyze_mask(mask)
    mpat = np.ascontiguousarray(pat_arr).astype(NPBF16)

    W_q = np.asarray(W_q, np.float32)
    W_k = np.asarray(W_k, np.float32)
    W_v = np.asarray(W_v, np.float32)
    W_o = np.asarray(W_o, np.float32)

    in_maps = []
    for c in range(N_CORES):
        cs = slice(c * CH, (c + 1) * CH)
        in_maps.append(
            {
                "xq": xq,
                "xk": xk,
                "xv": xv,
                "wq": np.ascontiguousarray(W_q[cs, :].T).astype(NPBF16),
                "wk": np.ascontiguousarray(W_k[cs, :].T).astype(NPBF16),
                "wv": np.ascontiguousarray(W_v[cs, :].T).astype(NPBF16),
                "wo": np.ascontiguousarray(W_o[:, cs].T).astype(NPBF16),
                "bq": np.asarray(b_q, np.float32)[cs].reshape(CH, 1).copy(),
                "bk": np.asarray(b_k, np.float32)[cs].reshape(CH, 1).copy(),
                "mpat": mpat,
            }
        )
    return plan, mpat.shape[1], in_maps


def run(inputs, trace=False, trace_cores=None):
    """Build (cached), run on 8 cores, return (final_output, BassKernelResults)."""
    plan, pat_w, in_maps = _prep_inputs(**inputs)
    nc = _get_module(plan, pat_w)
    res = bass_utils.run_bass_kernel_spmd(
        nc,
        in_maps,
        core_ids=list(range(N_CORES)),
        trace=trace,
        trace_cores=trace_cores,
    )
    acc = np.zeros((MO, B * NJ, P, CW), np.float32)
    for c in range(N_CORES):
        acc += res.results[c]["out"].astype(np.float32)
    acc = acc.transpose(0, 2, 1, 3).reshape(D, TOK)
    # v-bias contributes W_o @ b_v to every token; fold it into the out bias
    bo_eff = np.asarray(inputs["b_o"], np.float32) + np.asarray(
        inputs["W_o"], np.float32
    ) @ np.asarray(inputs["b_v"], np.float32)
    final = acc.T + bo_eff[None, :]
    return final.reshape(B, S, D), res


def kernel(**inputs):
    return run(inputs, trace=False)[0]


# revision 31
# speedup vs baseline: 1.0210x; 1.0210x over previous
"""Multi-head attention (B=2, S=2048, D=1024, H=16 heads, causal) on 8 trn2 cores.

Sharding: heads across cores (2 heads = 128 channels per core).
  - W_q/W_k/W_v column-sharded: each core projects all tokens to its 128 channels.
  - Attention per (batch, head) fully local to a core.
  - W_o row-sharded: each core computes a partial output projection; partials
    are summed on the host (the unshard step), then b_o (+ W_o @ b_v) is added.

Device layout: Q/K transposed (channels on partitions, tokens on free).
  - Scores computed as S^T blocks [128 k-tok, 512 q-tok] so exp is elementwise.
  - V^T produced directly by the projection (lhsT=x chunk, rhs=W_v chunk), no
    PE transposes. Each (b, head) V block carries 64 all-ones columns, so the
    AV matmul emits the softmax sums pre-broadcast across 64 partitions; the
    normalize is then a single tensor-tensor divide per head.
  - Causal structure: scores/exp/AV restricted to the valid q-range per
    k-block; the 127-wide diagonal triangle is multiplied in on GpSimd.
  - Projections / V^T pieces / output-projection pieces are emitted as filler
    between attention blocks so the PE never idles (keeps the 2.4 GHz p-state)
    while the scalar engine runs the exp stream.

All matmuls run in bf16 (inputs cast on host) with fp32 PSUM accumulation;
the partial output is returned bf16 and reduced in fp32 on the host.
"""

import sys
from collections import deque
from functools import partial

import numpy as np

try:
    import concourse.bass as bass  # noqa: F401
except ImportError:  # pragma: no cover
    sys.path.insert(0, "/opt/trn_rl_repo")

import ml_dtypes

import concourse.mybir as mybir
import concourse.tile as tile
from concourse import bacc, bass_utils
from concourse.masks import make_identity

P = 128
B, S, D = 2, 2048, 1024
H, DK = 16, 64
N_CORES = 8
HPC = H // N_CORES  # heads per core = 2
CH = HPC * DK  # channels per core = 128
TOK = B * S  # 4096
NKB = S // P  # k-blocks per batch = 16
CW = 512  # q column width
NJ = S // CW  # q columns per batch = 4
NTG = S // CW  # 512-token projection groups per batch = 4
KPG = CW // P  # k-blocks per token group = 4
XC = D // P  # x-dim chunks = 8
MO = D // P  # output-channel chunks = 8

BF16 = mybir.dt.bfloat16
F32 = mybir.dt.float32
NPBF16 = ml_dtypes.bfloat16

_BUILD_CACHE = {}


def _analyze_mask(mask):
    """Block plan from the (1,1,S,S) boolean mask (shared across batch/head).

    plan[j] = tuple of (bk, qa, mixed) for each k-block with any valid entry:
      qa    = first local q with any valid k; scores/exp/AV cover [qa, CW).
      mixed = None or (pat_off, a, w): a2[:, :, a:a+w] *= pattern columns.
    Patterns are deduplicated and concatenated into pats (P, W_total) in
    [k, q] layout.
    """
    m = np.asarray(mask).reshape(S, S).astype(bool)  # m[q, k]
    pat_index = {}
    pat_list = []
    plan = []
    for j in range(NJ):
        q0 = j * CW
        blocks = []
        first = True
        for bk in range(NKB):
            sub = m[q0 : q0 + CW, bk * P : (bk + 1) * P]  # (CW q, P k)
            anyv = sub.any(axis=1)
            if not anyv.any():
                continue
            qa = int(np.argmax(anyv))
            if first:
                # the first block initializes the whole PSUM accumulator
                qa = 0
                first = False
            validall = sub.all(axis=1)
            nfv = ~validall
            nfv[:qa] = False
            mixed = None
            if nfv.any():
                idx = np.where(nfv)[0]
                a_, b_ = int(idx[0]), int(idx[-1]) + 1
                patt = np.ascontiguousarray(sub[a_:b_, :].T).astype(np.float32)
                key = (patt.shape[1], patt.tobytes())
                if key not in pat_index:
                    pat_index[key] = len(pat_list)
                    pat_list.append(patt)
                mixed = (pat_index[key], a_, b_ - a_)
            blocks.append((bk, qa, mixed))
        plan.append(tuple(blocks))
    offs = [0]
    for p_ in pat_list:
        offs.append(offs[-1] + p_.shape[1])
    plan2 = []
    for col in plan:
        col2 = []
        for bk, qa, mixed in col:
            if mixed is not None:
                pid, a_, w_ = mixed
                mixed = (offs[pid], a_, w_)
            col2.append((bk, qa, mixed))
        plan2.append(tuple(col2))
    if pat_list:
        pat_arr = np.concatenate(pat_list, axis=1)  # (P, W_total)
    else:
        pat_arr = np.ones((P, 1), np.float32)
    return tuple(plan2), pat_arr


def _build(plan, pat_w):
    nc = bacc.Bacc(
        "TRN2",
        target_bir_lowering=False,
        debug=False,
        enable_asserts=True,
        num_devices=N_CORES,
    )
    NTT = B * NTG
    xq = nc.dram_tensor("xq", [NTT, P, XC, CW], BF16, kind="ExternalInput").ap()
    xk = nc.dram_tensor("xk", [NTT, P, XC, CW], BF16, kind="ExternalInput").ap()
    xv = nc.dram_tensor("xv", [NTT, P, XC, CW], BF16, kind="ExternalInput").ap()
    wq = nc.dram_tensor("wq", [D, CH], BF16, kind="ExternalInput").ap()
    wk = nc.dram_tensor("wk", [D, CH], BF16, kind="ExternalInput").ap()
    wv = nc.dram_tensor("wv", [D, CH], BF16, kind="ExternalInput").ap()
    wo = nc.dram_tensor("wo", [CH, D], BF16, kind="ExternalInput").ap()
    bq = nc.dram_tensor("bq", [CH, 1], F32, kind="ExternalInput").ap()
    bk_ = nc.dram_tensor("bk", [CH, 1], F32, kind="ExternalInput").ap()
    mpat = nc.dram_tensor("mpat", [P, pat_w], BF16, kind="ExternalInput").ap()
    out = nc.dram_tensor(
        "out", [MO, B * NJ, P, CW], BF16, kind="ExternalOutput"
    ).ap()

    xdram = {"q": xq, "k": xk, "v": xv}

    with tile.TileContext(nc) as tc:
        with (
            tc.tile_pool(name="const", bufs=1) as const,
            tc.tile_pool(name="persist", bufs=1) as persist,
            tc.tile_pool(name="xt", bufs=2) as xtp,
            tc.tile_pool(name="vt", bufs=2) as vtp,
            tc.tile_pool(name="a2", bufs=4) as a2p,
            tc.tile_pool(name="nrm", bufs=2) as nrm,
            tc.tile_pool(name="yt", bufs=3) as ytp,
            tc.tile_pool(name="ob", bufs=10) as obp,
            tc.tile_pool(name="pp", bufs=2, space="PSUM") as pp,
            tc.tile_pool(name="s2", bufs=2, space="PSUM") as s2p,
            tc.tile_pool(name="op", bufs=2, space="PSUM") as opsp,
        ):
            # --- constants: loaded via the Activation HWDGE queue so the SP
            # queue is dedicated to x prefetch ------------------------------
            w_sb = {}
            b_sb = {}
            for name, wdram, bdram in (
                ("q", wq, bq),
                ("k", wk, bk_),
                ("v", wv, None),
            ):
                w_sb[name] = const.tile(
                    [P, XC, CH], BF16, tag=f"w{name}", name=f"w{name}"
                )
                wview = wdram.rearrange("(o p) c -> p o c", p=P)
                for h in range(0, XC, 2):
                    nc.scalar.dma_start(
                        w_sb[name][:, h : h + 2, :], wview[:, h : h + 2, :]
                    )
                if bdram is not None:
                    b_sb[name] = const.tile(
                        [CH, 1], F32, tag=f"b{name}", name=f"b{name}"
                    )
                    nc.scalar.dma_start(b_sb[name][:], bdram)
            wo_sb = const.tile([CH, D], BF16, tag="wo")
            for h in range(0, D, CW):
                nc.scalar.dma_start(wo_sb[:, h : h + CW], wo[:, h : h + CW])
            mask_sb = const.tile([P, pat_w], BF16, tag="mpat")
            nc.scalar.dma_start(mask_sb[:], mpat)
            ident = const.tile([P, P], BF16, tag="ident")
            make_identity(nc, ident)

            # persistent Q^T/K^T [chan, tok] and V^T-augmented per batch
            qt, kt = {}, {}
            for b in range(B):
                qt[b] = persist.tile([CH, S], BF16, tag=f"qt{b}", name=f"qt{b}")
                kt[b] = persist.tile([CH, S], BF16, tag=f"kt{b}", name=f"kt{b}")
            # vaug[b]: [k-tok, NKB, head, 64 V cols + 64 ones cols]; the ones
            # make the AV matmul emit softmax sums broadcast on rows 64..127
            vaug = {}
            for b in range(B):
                vaug[b] = persist.tile(
                    [P, NKB, HPC, P], BF16, tag=f"vaug{b}", name=f"vaug{b}"
                )
                # contiguous fill; the V columns are overwritten by the V^T
                # evacs, leaving the ones columns that produce the sums rows
                nc.gpsimd.memset(vaug[b][:], 1.0)

            def load_x(b, tg, fine=False):
                """Prefetch x tiles; fine=True splits into 128KB chunks for a
                fast first matmul, otherwise 256KB halves the SP setup cost."""
                tiles = {}
                g = b * NTG + tg
                step = 1 if fine else 2
                for name in ("q", "k", "v"):
                    t = xtp.tile([P, XC, CW], BF16, tag=f"x{name}")
                    if fine and name == "q":
                        # halve the very first chunk so matmul 0 starts sooner
                        hw = CW // 2
                        nc.sync.dma_start(
                            t[:, 0, 0:hw], xdram[name][g, :, 0, 0:hw]
                        )
                        nc.sync.dma_start(
                            t[:, 0, hw:], xdram[name][g, :, 0, hw:]
                        )
                        first = 1
                    else:
                        first = 0
                    for xc in range(first, XC, step):
                        nc.sync.dma_start(
                            t[:, xc : xc + step, :],
                            xdram[name][g, :, xc : xc + step, :],
                        )
                    tiles[name] = t
                return tiles

            def emit_proj(b, name, tg, xtile):
                """Q/K projection of one 512-token group -> qt/kt columns."""
                ps = pp.tile([CH, CW], F32, tag="pp")
                for xc in range(XC):
                    nc.tensor.matmul(
                        ps[:],
                        lhsT=w_sb[name][:, xc, :],
                        rhs=xtile[:, xc, :],
                        start=(xc == 0),
                        stop=(xc == XC - 1),
                    )
                dst = qt if name == "q" else kt
                nc.vector.tensor_add(
                    dst[b][:, tg * CW : (tg + 1) * CW],
                    ps[:],
                    b_sb[name][:, 0:1].to_broadcast((CH, CW)),
                )

            def emit_proj_v(b, tg, xtile, vt_sb):
                """V projection of one token group in [chan, tok] layout."""
                ps = pp.tile([CH, CW], F32, tag="pp")
                for xc in range(XC):
                    nc.tensor.matmul(
                        ps[:],
                        lhsT=w_sb["v"][:, xc, :],
                        rhs=xtile[:, xc, :],
                        start=(xc == 0),
                        stop=(xc == XC - 1),
                    )
                nc.vector.tensor_copy(vt_sb[:], ps[:])

            def vt_unit(b, tg, tb, vt_sb):
                """Transpose one 128-token block of V into vaug [tok, chan]."""
                tp_ = pp.tile([P, HPC, DK], BF16, tag="pp")
                nc.tensor.transpose(
                    tp_[:], vt_sb[:, tb * P : (tb + 1) * P], ident[:]
                )
                kb = tg * KPG + tb
                nc.vector.tensor_copy(vaug[b][:, kb, :, 0:DK], tp_[:])

            def oproj_piece(tcol, yt, mo, last=False):
                op_ps = pp.tile([P, CW], F32, tag="pp")
                nc.tensor.matmul(
                    op_ps[:],
                    lhsT=wo_sb[:, mo * P : (mo + 1) * P],
                    rhs=yt[:],
                    start=True,
                    stop=True,
                )
                ob = obp.tile([P, CW], BF16, tag="ob")
                # evac split 2:6 ACT:DVE (the last column alternates evenly
                # and splits DMAs to shorten the kernel tail)
                on_act = mo % 2 == 0 if last else mo % 4 == 0
                if on_act:
                    nc.scalar.copy(ob[:], op_ps[:])
                else:
                    nc.vector.tensor_copy(ob[:], op_ps[:])
                if last:
                    # split across the SP and Pool DGE queues for a short tail
                    h = CW // 2
                    nc.sync.dma_start(out[mo, tcol][:, 0:h], ob[:, 0:h])
                    nc.gpsimd.dma_start(out[mo, tcol][:, h:], ob[:, h:])
                else:
                    nc.sync.dma_start(out[mo, tcol], ob[:])

            def attention_col(b, j, tg, vt_units, nxt_units, lazy, pace=450):
                """One 512-wide q column; drains filler units between blocks."""
                blocks = plan[j]
                q0 = j * CW
                yt = ytp.tile([CH, CW], BF16, tag="yt")
                if not blocks:
                    while vt_units:
                        _, fn = vt_units.popleft()
                        fn()
                    nc.gpsimd.memset(yt[:], 0.0)
                    return yt
                ops = [
                    opsp.tile([P, CW], F32, tag="op", name=f"op{hl}")
                    for hl in range(HPC)
                ]
                nblk = len(blocks)

                def emit_av(i, bk, qa, a2):
                    for hl in range(HPC):
                        nc.tensor.matmul(
                            ops[hl][:, qa:],
                            lhsT=vaug[b][:, bk, hl, :],
                            rhs=a2[:, hl, qa:],
                            start=(i == 0),
                            stop=(i == nblk - 1),
                            skip_group_check=True,
                        )

                pend = None
                debt = 0
                for i, (bk, qa, mixed) in enumerate(blocks):
                    k0 = bk * P
                    s2 = s2p.tile([P, HPC, CW], F32, tag="s2")
                    for hl in range(HPC):
                        hs = slice(hl * DK, (hl + 1) * DK)
                        nc.tensor.matmul(
                            s2[:, hl, qa:],
                            lhsT=kt[b][hs, k0 : k0 + P],
                            rhs=qt[b][hs, q0 + qa : q0 + CW],
                            start=True,
                            stop=True,
                            skip_group_check=True,
                        )
                    a2 = a2p.tile([P, HPC, CW], BF16, tag="a2")
                    nc.scalar.activation(
                        a2[:, :, qa:],
                        s2[:, :, qa:],
                        mybir.ActivationFunctionType.Exp,
                        scale=0.125,
                    )
                    if mixed is not None:
                        off, a_, w_ = mixed
                        nc.gpsimd.tensor_tensor(
                            a2[:, :, a_ : a_ + w_],
                            a2[:, :, a_ : a_ + w_],
                            mask_sb[:, None, off : off + w_].to_broadcast(
                                (P, HPC, w_)
                            ),
                            mybir.AluOpType.mult,
                        )
                    if pend is not None:
                        # this tg's V^T blocks must land before its AVs
                        if pend[1] >= tg * KPG:
                            while vt_units:
                                _, fn = vt_units.popleft()
                                fn()
                        emit_av(*pend)
                    pend = (i, bk, qa, a2)
                    # filler: keep the PE fed while the scalar engine exps
                    debt += pace
                    while debt > 0 and (vt_units or nxt_units or lazy):
                        src = (
                            vt_units
                            if vt_units
                            else (nxt_units if nxt_units else lazy)
                        )
                        ns, fn = src.popleft()
                        fn()
                        debt -= ns
                if pend[1] >= tg * KPG:
                    while vt_units:
                        _, fn = vt_units.popleft()
                        fn()
                emit_av(*pend)
                # normalize: rows 64..127 of ops are the sums, pre-broadcast.
                # DVE reads at most one PSUM operand per op, so reciprocal the
                # sums into SBUF, then multiply against the PSUM values.
                for hl in range(HPC):
                    # reciprocal_approx_fast misreads PSUM on HW — bounce the
                    # sums through SBUF (on ACT, to shorten the DVE chain)
                    sums = nrm.tile([DK, CW], F32, tag="sums", name=f"sums{hl}")
                    nc.scalar.copy(sums[:], ops[hl][DK:P, :])
                    rec = nrm.tile([DK, CW], F32, tag="rec", name=f"rec{hl}")
                    nc.vector.reciprocal_approx_fast(out=rec[:], in_=sums[:])
                    nc.vector.tensor_tensor(
                        yt[hl * DK : (hl + 1) * DK, :],
                        ops[hl][0:DK, :],
                        rec[:],
                        mybir.AluOpType.mult,
                    )
                return yt

            # --- main schedule ---------------------------------------------
            seq = [(b, tg) for b in range(B) for tg in range(NTG)]
            cur_x = load_x(*seq[0], fine=True)
            lazy = deque()
            for si, (b, tg) in enumerate(seq):
                nxt_x = load_x(*seq[si + 1]) if si + 1 < len(seq) else None
                emit_proj(b, "q", tg, cur_x["q"])
                emit_proj(b, "k", tg, cur_x["k"])
                vt_sb = vtp.tile([CH, CW], BF16, tag="vt")
                emit_proj_v(b, tg, cur_x["v"], vt_sb)
                vt_units = deque(
                    (450, partial(vt_unit, b, tg, tb, vt_sb))
                    for tb in range(KPG)
                )
                last = si == len(seq) - 1
                yt = attention_col(
                    b, tg, tg, vt_units, deque(), lazy,
                    pace=150 if last else 600,
                )
                while vt_units:
                    _, fn = vt_units.popleft()
                    fn()
                tcol = b * NJ + tg
                lazy.extend(
                    (450, partial(oproj_piece, tcol, yt, mo, last))
                    for mo in range(MO)
                )
                cur_x = nxt_x
            while lazy:
                _, fn = lazy.popleft()
                fn()
    nc.compile()
    return nc


def _get_module(plan, pat_w):
    key = (plan, pat_w)
    if key not in _BUILD_CACHE:
        _BUILD_CACHE[key] = _build(plan, pat_w)
    return _BUILD_CACHE[key]


def _prep_inputs(query, key, value, mask, W_q, b_q, W_k, b_k, W_v, b_v, W_o, b_o):
    def xt_of(x):
        x2 = np.asarray(x, np.float32).reshape(TOK, D)
        xt = x2.T.astype(NPBF16)  # (D, TOK)
        xt = xt.reshape(XC, P, B * NTG, CW).transpose(2, 1, 0, 3)
        return np.ascontiguousarray(xt)  # (NTT, P, XC, CW)

    xq, xk, xv = xt_of(query), xt_of(key), xt_of(value)
    plan, pat_arr = _analyze_mask(mask)
    mpat = np.ascontiguousarray(pat_arr).astype(NPBF16)

    W_q = np.asarray(W_q, np.float32)
    W_k = np.asarray(W_k, np.float32)
    W_v = np.asarray(W_v, np.float32)
    W_o = np.asarray(W_o, np.float32)

    in_maps = []
    for c in range(N_CORES):
        cs = slice(c * CH, (c + 1) * CH)
        in_maps.append(
            {
                "xq": xq,
                "xk": xk,
                "xv": xv,
                "wq": np.ascontiguousarray(W_q[cs, :].T).astype(NPBF16),
                "wk": np.ascontiguousarray(W_k[cs, :].T).astype(NPBF16),
                "wv": np.ascontiguousarray(W_v[cs, :].T).astype(NPBF16),
                "wo": np.ascontiguousarray(W_o[:, cs].T).astype(NPBF16),
                "bq": np.asarray(b_q, np.float32)[cs].reshape(CH, 1).copy(),
                "bk": np.asarray(b_k, np.float32)[cs].reshape(CH, 1).copy(),
                "mpat": mpat,
            }
        )
    return plan, mpat.shape[1], in_maps


def run(inputs, trace=False, trace_cores=None):
    """Build (cached), run on 8 cores, return (final_output, BassKernelResults)."""
    plan, pat_w, in_maps = _prep_inputs(**inputs)
    nc = _get_module(plan, pat_w)
    res = bass_utils.run_bass_kernel_spmd(
        nc,
        in_maps,
        core_ids=list(range(N_CORES)),
        trace=trace,
        trace_cores=trace_cores,
    )
    acc = np.zeros((MO, B * NJ, P, CW), np.float32)
    for c in range(N_CORES):
        acc += res.results[c]["out"].astype(np.float32)
    acc = acc.transpose(0, 2, 1, 3).reshape(D, TOK)
    # v-bias contributes W_o @ b_v to every token; fold it into the out bias
    bo_eff = np.asarray(inputs["b_o"], np.float32) + np.asarray(
        inputs["W_o"], np.float32
    ) @ np.asarray(inputs["b_v"], np.float32)
    final = acc.T + bo_eff[None, :]
    return final.reshape(B, S, D), res


def kernel(**inputs):
    return run(inputs, trace=False)[0]


# revision 32
# speedup vs baseline: 1.0483x; 1.0266x over previous
"""Multi-head attention (B=2, S=2048, D=1024, H=16 heads, causal) on 8 trn2 cores.

Sharding: heads across cores (2 heads = 128 channels per core).
  - W_q/W_k/W_v column-sharded: each core projects all tokens to its 128 channels.
  - Attention per (batch, head) fully local to a core.
  - W_o row-sharded: each core computes a partial output projection; partials
    are summed on the host (the unshard step), then b_o (+ W_o @ b_v) is added.

Device layout: Q/K transposed (channels on partitions, tokens on free).
  - Scores computed as S^T blocks [128 k-tok, 512 q-tok] so exp is elementwise.
  - V^T produced directly by the projection (lhsT=x chunk, rhs=W_v chunk), no
    PE transposes. Each (b, head) V block carries 64 all-ones columns, so the
    AV matmul emits the softmax sums pre-broadcast across 64 partitions; the
    normalize is then a single tensor-tensor divide per head.
  - Causal structure: scores/exp/AV restricted to the valid q-range per
    k-block; the 127-wide diagonal triangle is multiplied in on GpSimd.
  - Projections / V^T pieces / output-projection pieces are emitted as filler
    between attention blocks so the PE never idles (keeps the 2.4 GHz p-state)
    while the scalar engine runs the exp stream.

All matmuls run in bf16 (inputs cast on host) with fp32 PSUM accumulation;
the partial output is returned bf16 and reduced in fp32 on the host.
"""

import sys
from collections import deque
from functools import partial

import numpy as np

try:
    import concourse.bass as bass  # noqa: F401
except ImportError:  # pragma: no cover
    sys.path.insert(0, "/opt/trn_rl_repo")

import ml_dtypes

import concourse.mybir as mybir
import concourse.tile as tile
from concourse import bacc, bass_utils
from concourse.masks import make_identity

P = 128
B, S, D = 2, 2048, 1024
H, DK = 16, 64
N_CORES = 8
HPC = H // N_CORES  # heads per core = 2
CH = HPC * DK  # channels per core = 128
TOK = B * S  # 4096
NKB = S // P  # k-blocks per batch = 16
CW = 512  # q column width
NJ = S // CW  # q columns per batch = 4
NTG = S // CW  # 512-token projection groups per batch = 4
KPG = CW // P  # k-blocks per token group = 4
XC = D // P  # x-dim chunks = 8
MO = D // P  # output-channel chunks = 8

BF16 = mybir.dt.bfloat16
F32 = mybir.dt.float32
NPBF16 = ml_dtypes.bfloat16

_BUILD_CACHE = {}


def _analyze_mask(mask):
    """Block plan from the (1,1,S,S) boolean mask (shared across batch/head).

    plan[j] = tuple of (bk, qa, mixed) for each k-block with any valid entry:
      qa    = first local q with any valid k; scores/exp/AV cover [qa, CW).
      mixed = None or (pat_off, a, w): a2[:, :, a:a+w] *= pattern columns.
    Patterns are deduplicated and concatenated into pats (P, W_total) in
    [k, q] layout.
    """
    m = np.asarray(mask).reshape(S, S).astype(bool)  # m[q, k]
    pat_index = {}
    pat_list = []
    plan = []
    for j in range(NJ):
        q0 = j * CW
        blocks = []
        first = True
        for bk in range(NKB):
            sub = m[q0 : q0 + CW, bk * P : (bk + 1) * P]  # (CW q, P k)
            anyv = sub.any(axis=1)
            if not anyv.any():
                continue
            qa = int(np.argmax(anyv))
            if first:
                # the first block initializes the whole PSUM accumulator
                qa = 0
                first = False
            validall = sub.all(axis=1)
            nfv = ~validall
            nfv[:qa] = False
            mixed = None
            if nfv.any():
                idx = np.where(nfv)[0]
                a_, b_ = int(idx[0]), int(idx[-1]) + 1
                patt = np.ascontiguousarray(sub[a_:b_, :].T).astype(np.float32)
                key = (patt.shape[1], patt.tobytes())
                if key not in pat_index:
                    pat_index[key] = len(pat_list)
                    pat_list.append(patt)
                mixed = (pat_index[key], a_, b_ - a_)
            blocks.append((bk, qa, mixed))
        plan.append(tuple(blocks))
    offs = [0]
    for p_ in pat_list:
        offs.append(offs[-1] + p_.shape[1])
    plan2 = []
    for col in plan:
        col2 = []
        for bk, qa, mixed in col:
            if mixed is not None:
                pid, a_, w_ = mixed
                mixed = (offs[pid], a_, w_)
            col2.append((bk, qa, mixed))
        plan2.append(tuple(col2))
    if pat_list:
        pat_arr = np.concatenate(pat_list, axis=1)  # (P, W_total)
    else:
        pat_arr = np.ones((P, 1), np.float32)
    return tuple(plan2), pat_arr


def _build(plan, pat_w):
    nc = bacc.Bacc(
        "TRN2",
        target_bir_lowering=False,
        debug=False,
        enable_asserts=True,
        num_devices=N_CORES,
    )
    NTT = B * NTG
    xq = nc.dram_tensor("xq", [NTT, P, XC, CW], BF16, kind="ExternalInput").ap()
    xk = nc.dram_tensor("xk", [NTT, P, XC, CW], BF16, kind="ExternalInput").ap()
    xv = nc.dram_tensor("xv", [NTT, P, XC, CW], BF16, kind="ExternalInput").ap()
    wq = nc.dram_tensor("wq", [D, CH], BF16, kind="ExternalInput").ap()
    wk = nc.dram_tensor("wk", [D, CH], BF16, kind="ExternalInput").ap()
    wv = nc.dram_tensor("wv", [D, CH], BF16, kind="ExternalInput").ap()
    wo = nc.dram_tensor("wo", [CH, D], BF16, kind="ExternalInput").ap()
    bq = nc.dram_tensor("bq", [CH, 1], F32, kind="ExternalInput").ap()
    bk_ = nc.dram_tensor("bk", [CH, 1], F32, kind="ExternalInput").ap()
    mpat = nc.dram_tensor("mpat", [P, pat_w], BF16, kind="ExternalInput").ap()
    out = nc.dram_tensor(
        "out", [MO, B * NJ, P, CW], BF16, kind="ExternalOutput"
    ).ap()

    xdram = {"q": xq, "k": xk, "v": xv}

    with tile.TileContext(nc) as tc:
        with (
            tc.tile_pool(name="const", bufs=1) as const,
            tc.tile_pool(name="persist", bufs=1) as persist,
            tc.tile_pool(name="xt", bufs=2) as xtp,
            tc.tile_pool(name="vt", bufs=2) as vtp,
            tc.tile_pool(name="a2", bufs=4) as a2p,
            tc.tile_pool(name="nrm", bufs=2) as nrm,
            tc.tile_pool(name="yt", bufs=3) as ytp,
            tc.tile_pool(name="ob", bufs=10) as obp,
            tc.tile_pool(name="pp", bufs=2, space="PSUM") as pp,
            tc.tile_pool(name="s2", bufs=2, space="PSUM") as s2p,
            tc.tile_pool(name="op", bufs=2, space="PSUM") as opsp,
        ):
            # --- constants: loaded via the Activation HWDGE queue so the SP
            # queue is dedicated to x prefetch ------------------------------
            w_sb = {}
            b_sb = {}
            for name, wdram, bdram in (
                ("q", wq, bq),
                ("k", wk, bk_),
                ("v", wv, None),
            ):
                w_sb[name] = const.tile(
                    [P, XC, CH], BF16, tag=f"w{name}", name=f"w{name}"
                )
                wview = wdram.rearrange("(o p) c -> p o c", p=P)
                for h in range(0, XC, 2):
                    nc.scalar.dma_start(
                        w_sb[name][:, h : h + 2, :], wview[:, h : h + 2, :]
                    )
                if bdram is not None:
                    b_sb[name] = const.tile(
                        [CH, 1], F32, tag=f"b{name}", name=f"b{name}"
                    )
                    nc.scalar.dma_start(b_sb[name][:], bdram)
            wo_sb = const.tile([CH, D], BF16, tag="wo")
            for h in range(0, D, CW):
                nc.scalar.dma_start(wo_sb[:, h : h + CW], wo[:, h : h + CW])
            mask_sb = const.tile([P, pat_w], BF16, tag="mpat")
            nc.scalar.dma_start(mask_sb[:], mpat)
            ident = const.tile([P, P], BF16, tag="ident")
            make_identity(nc, ident)

            # persistent Q^T/K^T [chan, tok] and V^T-augmented per batch
            qt, kt = {}, {}
            for b in range(B):
                qt[b] = persist.tile([CH, S], BF16, tag=f"qt{b}", name=f"qt{b}")
                kt[b] = persist.tile([CH, S], BF16, tag=f"kt{b}", name=f"kt{b}")
            # vaug[b]: [k-tok, NKB, head, 64 V cols + 64 ones cols]; the ones
            # make the AV matmul emit softmax sums broadcast on rows 64..127
            vaug = {}
            for b in range(B):
                vaug[b] = persist.tile(
                    [P, NKB, HPC, P], BF16, tag=f"vaug{b}", name=f"vaug{b}"
                )
                # contiguous fill; the V columns are overwritten by the V^T
                # evacs, leaving the ones columns that produce the sums rows
                nc.gpsimd.memset(vaug[b][:], 1.0)

            def load_x(b, tg, fine=False):
                """Prefetch x tiles; fine=True splits into 128KB chunks for a
                fast first matmul, otherwise 256KB halves the SP setup cost."""
                tiles = {}
                g = b * NTG + tg
                step = 1 if fine else 2
                for name in ("q", "k", "v"):
                    t = xtp.tile([P, XC, CW], BF16, tag=f"x{name}")
                    if fine and name == "q":
                        # halve the very first chunk so matmul 0 starts sooner
                        hw = CW // 2
                        nc.sync.dma_start(
                            t[:, 0, 0:hw], xdram[name][g, :, 0, 0:hw]
                        )
                        nc.sync.dma_start(
                            t[:, 0, hw:], xdram[name][g, :, 0, hw:]
                        )
                        first = 1
                    else:
                        first = 0
                    for xc in range(first, XC, step):
                        nc.sync.dma_start(
                            t[:, xc : xc + step, :],
                            xdram[name][g, :, xc : xc + step, :],
                        )
                    tiles[name] = t
                return tiles

            def emit_proj(b, name, tg, xtile):
                """Q/K projection of one 512-token group -> qt/kt columns."""
                ps = pp.tile([CH, CW], F32, tag="pp")
                for xc in range(XC):
                    nc.tensor.matmul(
                        ps[:],
                        lhsT=w_sb[name][:, xc, :],
                        rhs=xtile[:, xc, :],
                        start=(xc == 0),
                        stop=(xc == XC - 1),
                    )
                dst = qt if name == "q" else kt
                nc.vector.tensor_add(
                    dst[b][:, tg * CW : (tg + 1) * CW],
                    ps[:],
                    b_sb[name][:, 0:1].to_broadcast((CH, CW)),
                )

            def emit_proj_v(b, tg, xtile, vt_sb):
                """V projection of one token group in [chan, tok] layout."""
                ps = pp.tile([CH, CW], F32, tag="pp")
                for xc in range(XC):
                    nc.tensor.matmul(
                        ps[:],
                        lhsT=w_sb["v"][:, xc, :],
                        rhs=xtile[:, xc, :],
                        start=(xc == 0),
                        stop=(xc == XC - 1),
                    )
                nc.vector.tensor_copy(vt_sb[:], ps[:])

            def vt_unit(b, tg, tb, vt_sb):
                """Transpose one 128-token block of V into vaug [tok, chan]."""
                tp_ = pp.tile([P, HPC, DK], BF16, tag="pp")
                nc.tensor.transpose(
                    tp_[:], vt_sb[:, tb * P : (tb + 1) * P], ident[:]
                )
                kb = tg * KPG + tb
                nc.vector.tensor_copy(vaug[b][:, kb, :, 0:DK], tp_[:])

            def oproj_piece(tcol, yt, mo, last=False):
                op_ps = pp.tile([P, CW], F32, tag="pp")
                nc.tensor.matmul(
                    op_ps[:],
                    lhsT=wo_sb[:, mo * P : (mo + 1) * P],
                    rhs=yt[:],
                    start=True,
                    stop=True,
                )
                ob = obp.tile([P, CW], BF16, tag="ob")
                # evac split 2:6 ACT:DVE (the last column alternates evenly
                # and splits DMAs to shorten the kernel tail)
                on_act = mo % 2 == 0 if last else mo % 4 == 0
                if on_act:
                    nc.scalar.copy(ob[:], op_ps[:])
                else:
                    nc.vector.tensor_copy(ob[:], op_ps[:])
                if last:
                    # split across the SP and Pool DGE queues for a short tail
                    h = CW // 2
                    nc.sync.dma_start(out[mo, tcol][:, 0:h], ob[:, 0:h])
                    nc.gpsimd.dma_start(out[mo, tcol][:, h:], ob[:, h:])
                else:
                    nc.sync.dma_start(out[mo, tcol], ob[:])

            def attention_col(b, j, tg, vt_units, nxt_units, lazy, pace=450):
                """One 512-wide q column; drains filler units between blocks."""
                blocks = plan[j]
                q0 = j * CW
                yt = ytp.tile([CH, CW], BF16, tag="yt")
                if not blocks:
                    while vt_units:
                        _, fn = vt_units.popleft()
                        fn()
                    nc.gpsimd.memset(yt[:], 0.0)
                    return yt
                ops = [
                    opsp.tile([P, CW], F32, tag="op", name=f"op{hl}")
                    for hl in range(HPC)
                ]
                nblk = len(blocks)

                def emit_av(i, bk, qa, a2):
                    for hl in range(HPC):
                        nc.tensor.matmul(
                            ops[hl][:, qa:],
                            lhsT=vaug[b][:, bk, hl, :],
                            rhs=a2[:, hl, qa:],
                            start=(i == 0),
                            stop=(i == nblk - 1),
                            skip_group_check=True,
                        )

                pend = None
                debt = 0
                for i, (bk, qa, mixed) in enumerate(blocks):
                    k0 = bk * P
                    s2 = s2p.tile([P, HPC, CW], F32, tag="s2")
                    for hl in range(HPC):
                        hs = slice(hl * DK, (hl + 1) * DK)
                        nc.tensor.matmul(
                            s2[:, hl, qa:],
                            lhsT=kt[b][hs, k0 : k0 + P],
                            rhs=qt[b][hs, q0 + qa : q0 + CW],
                            start=True,
                            stop=True,
                            skip_group_check=True,
                        )
                    a2 = a2p.tile([P, HPC, CW], BF16, tag="a2")
                    nc.scalar.activation(
                        a2[:, :, qa:],
                        s2[:, :, qa:],
                        mybir.ActivationFunctionType.Exp,
                        scale=0.125,
                    )
                    if mixed is not None:
                        off, a_, w_ = mixed
                        nc.gpsimd.tensor_tensor(
                            a2[:, :, a_ : a_ + w_],
                            a2[:, :, a_ : a_ + w_],
                            mask_sb[:, None, off : off + w_].to_broadcast(
                                (P, HPC, w_)
                            ),
                            mybir.AluOpType.mult,
                        )
                    if pend is not None:
                        # this tg's V^T blocks must land before its AVs
                        if pend[1] >= tg * KPG:
                            while vt_units:
                                _, fn = vt_units.popleft()
                                fn()
                        emit_av(*pend)
                    pend = (i, bk, qa, a2)
                    # filler: keep the PE fed while the scalar engine exps
                    debt += pace
                    while debt > 0 and (vt_units or nxt_units or lazy):
                        src = (
                            vt_units
                            if vt_units
                            else (nxt_units if nxt_units else lazy)
                        )
                        ns, fn = src.popleft()
                        fn()
                        debt -= ns
                if pend[1] >= tg * KPG:
                    while vt_units:
                        _, fn = vt_units.popleft()
                        fn()
                emit_av(*pend)
                # normalize: rows 64..127 of ops are the sums, pre-broadcast.
                # DVE reads at most one PSUM operand per op, so reciprocal the
                # sums into SBUF, then multiply against the PSUM values.
                for hl in range(HPC):
                    # reciprocal_approx_fast misreads PSUM on HW — bounce the
                    # sums through SBUF (on ACT, to shorten the DVE chain)
                    sums = nrm.tile([DK, CW], F32, tag="sums", name=f"sums{hl}")
                    nc.scalar.copy(sums[:], ops[hl][DK:P, :])
                    rec = nrm.tile([DK, CW], F32, tag="rec", name=f"rec{hl}")
                    nc.vector.reciprocal_approx_fast(out=rec[:], in_=sums[:])
                    nc.vector.tensor_tensor(
                        yt[hl * DK : (hl + 1) * DK, :],
                        ops[hl][0:DK, :],
                        rec[:],
                        mybir.AluOpType.mult,
                    )
                return yt

            # --- main schedule ---------------------------------------------
            seq = [(b, tg) for b in range(B) for tg in range(NTG)]
            cur_x = load_x(*seq[0], fine=True)
            lazy = deque()
            for si, (b, tg) in enumerate(seq):
                nxt_x = load_x(*seq[si + 1]) if si + 1 < len(seq) else None
                emit_proj(b, "q", tg, cur_x["q"])
                emit_proj(b, "k", tg, cur_x["k"])
                vt_sb = vtp.tile([CH, CW], BF16, tag="vt")
                emit_proj_v(b, tg, cur_x["v"], vt_sb)
                vt_units = deque(
                    (450, partial(vt_unit, b, tg, tb, vt_sb))
                    for tb in range(KPG)
                )
                last = si == len(seq) - 1
                yt = attention_col(
                    b, tg, tg, vt_units, deque(), lazy,
                    pace=150 if last else 450,
                )
                while vt_units:
                    _, fn = vt_units.popleft()
                    fn()
                tcol = b * NJ + tg
                lazy.extend(
                    (450, partial(oproj_piece, tcol, yt, mo, last))
                    for mo in range(MO)
                )
                cur_x = nxt_x
            while lazy:
                _, fn = lazy.popleft()
                fn()
    nc.compile()
    return nc


def _get_module(plan, pat_w):
    key = (plan, pat_w)
    if key not in _BUILD_CACHE:
        _BUILD_CACHE[key] = _build(plan, pat_w)
    return _BUILD_CACHE[key]


def _prep_inputs(query, key, value, mask, W_q, b_q, W_k, b_k, W_v, b_v, W_o, b_o):
    def xt_of(x):
        x2 = np.asarray(x, np.float32).reshape(TOK, D)
        xt = x2.T.astype(NPBF16)  # (D, TOK)
        xt = xt.reshape(XC, P, B * NTG, CW).transpose(2, 1, 0, 3)
        return np.ascontiguousarray(xt)  # (NTT, P, XC, CW)

    xq, xk, xv = xt_of(query), xt_of(key), xt_of(value)
    plan, pat_arr = _analyze_mask(mask)
    mpat = np.ascontiguousarray(pat_arr).astype(NPBF16)

    W_q = np.asarray(W_q, np.float32)
    W_k = np.asarray(W_k, np.float32)
    W_v = np.asarray(W_v, np.float32)
    W_o = np.asarray(W_o, np.float32)

    in_maps = []
    for c in range(N_CORES):
        cs = slice(c * CH, (c + 1) * CH)
        in_maps.append(
            {
                "xq": xq,
                "xk": xk,
                "xv": xv,
                "wq": np.ascontiguousarray(W_q[cs, :].T).astype(NPBF16),
                "wk": np.ascontiguousarray(W_k[cs, :].T).astype(NPBF16),
                "wv": np.ascontiguousarray(W_v[cs, :].T).astype(NPBF16),
                "wo": np.ascontiguousarray(W_o[:, cs].T).astype(NPBF16),
                "bq": np.asarray(b_q, np.float32)[cs].reshape(CH, 1).copy(),
                "bk": np.asarray(b_k, np.float32)[cs].reshape(CH, 1).copy(),
                "mpat": mpat,
            }
        )
    return plan, mpat.shape[1], in_maps


def run(inputs, trace=False, trace_cores=None):
    """Build (cached), run on 8 cores, return (final_output, BassKernelResults)."""
    plan, pat_w, in_maps = _prep_inputs(**inputs)
    nc = _get_module(plan, pat_w)
    res = bass_utils.run_bass_kernel_spmd(
        nc,
        in_maps,
        core_ids=list(range(N_CORES)),
        trace=trace,
        trace_cores=trace_cores,
    )
    acc = np.zeros((MO, B * NJ, P, CW), np.float32)
    for c in range(N_CORES):
        acc += res.results[c]["out"].astype(np.float32)
    acc = acc.transpose(0, 2, 1, 3).reshape(D, TOK)
    # v-bias contributes W_o @ b_v to every token; fold it into the out bias
    bo_eff = np.asarray(inputs["b_o"], np.float32) + np.asarray(
        inputs["W_o"], np.float32
    ) @ np.asarray(inputs["b_v"], np.float32)
    final = acc.T + bo_eff[None, :]
    return final.reshape(B, S, D), res


def kernel(**inputs):
    return run(inputs, trace=False)[0]


# revision 33
# speedup vs baseline: 1.0500x; 1.0017x over previous
"""Multi-head attention (B=2, S=2048, D=1024, H=16 heads, causal) on 8 trn2 cores.

Sharding: heads across cores (2 heads = 128 channels per core).
  - W_q/W_k/W_v column-sharded: each core projects all tokens to its 128 channels.
  - Attention per (batch, head) fully local to a core.
  - W_o row-sharded: each core computes a partial output projection; partials
    are summed on the host (the unshard step), then b_o (+ W_o @ b_v) is added.

Device layout: Q/K transposed (channels on partitions, tokens on free).
  - Scores computed as S^T blocks [128 k-tok, 512 q-tok] so exp is elementwise.
  - V^T produced directly by the projection (lhsT=x chunk, rhs=W_v chunk), no
    PE transposes. Each (b, head) V block carries 64 all-ones columns, so the
    AV matmul emits the softmax sums pre-broadcast across 64 partitions; the
    normalize is then a single tensor-tensor divide per head.
  - Causal structure: scores/exp/AV restricted to the valid q-range per
    k-block; the 127-wide diagonal triangle is multiplied in on GpSimd.
  - Projections / V^T pieces / output-projection pieces are emitted as filler
    between attention blocks so the PE never idles (keeps the 2.4 GHz p-state)
    while the scalar engine runs the exp stream.

All matmuls run in bf16 (inputs cast on host) with fp32 PSUM accumulation;
the partial output is returned bf16 and reduced in fp32 on the host.
"""

import sys
from collections import deque
from functools import partial

import numpy as np

try:
    import concourse.bass as bass  # noqa: F401
except ImportError:  # pragma: no cover
    sys.path.insert(0, "/opt/trn_rl_repo")

import ml_dtypes

import concourse.mybir as mybir
import concourse.tile as tile
from concourse import bacc, bass_utils
from concourse.masks import make_identity

P = 128
B, S, D = 2, 2048, 1024
H, DK = 16, 64
N_CORES = 8
HPC = H // N_CORES  # heads per core = 2
CH = HPC * DK  # channels per core = 128
TOK = B * S  # 4096
NKB = S // P  # k-blocks per batch = 16
CW = 512  # q column width
NJ = S // CW  # q columns per batch = 4
NTG = S // CW  # 512-token projection groups per batch = 4
KPG = CW // P  # k-blocks per token group = 4
XC = D // P  # x-dim chunks = 8
MO = D // P  # output-channel chunks = 8

BF16 = mybir.dt.bfloat16
F32 = mybir.dt.float32
NPBF16 = ml_dtypes.bfloat16

_BUILD_CACHE = {}


def _analyze_mask(mask):
    """Block plan from the (1,1,S,S) boolean mask (shared across batch/head).

    plan[j] = tuple of (bk, qa, mixed) for each k-block with any valid entry:
      qa    = first local q with any valid k; scores/exp/AV cover [qa, CW).
      mixed = None or (pat_off, a, w): a2[:, :, a:a+w] *= pattern columns.
    Patterns are deduplicated and concatenated into pats (P, W_total) in
    [k, q] layout.
    """
    m = np.asarray(mask).reshape(S, S).astype(bool)  # m[q, k]
    pat_index = {}
    pat_list = []
    plan = []
    for j in range(NJ):
        q0 = j * CW
        blocks = []
        first = True
        for bk in range(NKB):
            sub = m[q0 : q0 + CW, bk * P : (bk + 1) * P]  # (CW q, P k)
            anyv = sub.any(axis=1)
            if not anyv.any():
                continue
            qa = int(np.argmax(anyv))
            if first:
                # the first block initializes the whole PSUM accumulator
                qa = 0
                first = False
            validall = sub.all(axis=1)
            nfv = ~validall
            nfv[:qa] = False
            mixed = None
            if nfv.any():
                idx = np.where(nfv)[0]
                a_, b_ = int(idx[0]), int(idx[-1]) + 1
                patt = np.ascontiguousarray(sub[a_:b_, :].T).astype(np.float32)
                key = (patt.shape[1], patt.tobytes())
                if key not in pat_index:
                    pat_index[key] = len(pat_list)
                    pat_list.append(patt)
                mixed = (pat_index[key], a_, b_ - a_)
            blocks.append((bk, qa, mixed))
        plan.append(tuple(blocks))
    offs = [0]
    for p_ in pat_list:
        offs.append(offs[-1] + p_.shape[1])
    plan2 = []
    for col in plan:
        col2 = []
        for bk, qa, mixed in col:
            if mixed is not None:
                pid, a_, w_ = mixed
                mixed = (offs[pid], a_, w_)
            col2.append((bk, qa, mixed))
        plan2.append(tuple(col2))
    if pat_list:
        pat_arr = np.concatenate(pat_list, axis=1)  # (P, W_total)
    else:
        pat_arr = np.ones((P, 1), np.float32)
    return tuple(plan2), pat_arr


def _build(plan, pat_w):
    nc = bacc.Bacc(
        "TRN2",
        target_bir_lowering=False,
        debug=False,
        enable_asserts=True,
        num_devices=N_CORES,
    )
    NTT = B * NTG
    xq = nc.dram_tensor("xq", [NTT, P, XC, CW], BF16, kind="ExternalInput").ap()
    xk = nc.dram_tensor("xk", [NTT, P, XC, CW], BF16, kind="ExternalInput").ap()
    xv = nc.dram_tensor("xv", [NTT, P, XC, CW], BF16, kind="ExternalInput").ap()
    wq = nc.dram_tensor("wq", [D, CH], BF16, kind="ExternalInput").ap()
    wk = nc.dram_tensor("wk", [D, CH], BF16, kind="ExternalInput").ap()
    wv = nc.dram_tensor("wv", [D, CH], BF16, kind="ExternalInput").ap()
    wo = nc.dram_tensor("wo", [CH, D], BF16, kind="ExternalInput").ap()
    bq = nc.dram_tensor("bq", [CH, 1], F32, kind="ExternalInput").ap()
    bk_ = nc.dram_tensor("bk", [CH, 1], F32, kind="ExternalInput").ap()
    mpat = nc.dram_tensor("mpat", [P, pat_w], BF16, kind="ExternalInput").ap()
    out = nc.dram_tensor(
        "out", [MO, B * NJ, P, CW], BF16, kind="ExternalOutput"
    ).ap()

    xdram = {"q": xq, "k": xk, "v": xv}

    with tile.TileContext(nc) as tc:
        with (
            tc.tile_pool(name="const", bufs=1) as const,
            tc.tile_pool(name="persist", bufs=1) as persist,
            tc.tile_pool(name="xt", bufs=2) as xtp,
            tc.tile_pool(name="vt", bufs=3) as vtp,
            tc.tile_pool(name="a2", bufs=5) as a2p,
            tc.tile_pool(name="nrm", bufs=4) as nrm,
            tc.tile_pool(name="yt", bufs=4) as ytp,
            tc.tile_pool(name="ob", bufs=16) as obp,
            tc.tile_pool(name="pp", bufs=2, space="PSUM") as pp,
            tc.tile_pool(name="s2", bufs=2, space="PSUM") as s2p,
            tc.tile_pool(name="op", bufs=2, space="PSUM") as opsp,
        ):
            # --- constants: loaded via the Activation HWDGE queue so the SP
            # queue is dedicated to x prefetch ------------------------------
            w_sb = {}
            b_sb = {}
            for name, wdram, bdram in (
                ("q", wq, bq),
                ("k", wk, bk_),
                ("v", wv, None),
            ):
                w_sb[name] = const.tile(
                    [P, XC, CH], BF16, tag=f"w{name}", name=f"w{name}"
                )
                wview = wdram.rearrange("(o p) c -> p o c", p=P)
                for h in range(0, XC, 2):
                    nc.scalar.dma_start(
                        w_sb[name][:, h : h + 2, :], wview[:, h : h + 2, :]
                    )
                if bdram is not None:
                    b_sb[name] = const.tile(
                        [CH, 1], F32, tag=f"b{name}", name=f"b{name}"
                    )
                    nc.scalar.dma_start(b_sb[name][:], bdram)
            wo_sb = const.tile([CH, D], BF16, tag="wo")
            for h in range(0, D, CW):
                nc.scalar.dma_start(wo_sb[:, h : h + CW], wo[:, h : h + CW])
            mask_sb = const.tile([P, pat_w], BF16, tag="mpat")
            nc.scalar.dma_start(mask_sb[:], mpat)
            ident = const.tile([P, P], BF16, tag="ident")
            make_identity(nc, ident)

            # persistent Q^T/K^T [chan, tok] and V^T-augmented per batch
            qt, kt = {}, {}
            for b in range(B):
                qt[b] = persist.tile([CH, S], BF16, tag=f"qt{b}", name=f"qt{b}")
                kt[b] = persist.tile([CH, S], BF16, tag=f"kt{b}", name=f"kt{b}")
            # vaug[b]: [k-tok, NKB, head, 64 V cols + 64 ones cols]; the ones
            # make the AV matmul emit softmax sums broadcast on rows 64..127
            vaug = {}
            for b in range(B):
                vaug[b] = persist.tile(
                    [P, NKB, HPC, P], BF16, tag=f"vaug{b}", name=f"vaug{b}"
                )
                # contiguous fill; the V columns are overwritten by the V^T
                # evacs, leaving the ones columns that produce the sums rows
                nc.gpsimd.memset(vaug[b][:], 1.0)

            def load_x(b, tg, fine=False):
                """Prefetch x tiles; fine=True splits into 128KB chunks for a
                fast first matmul, otherwise 256KB halves the SP setup cost."""
                tiles = {}
                g = b * NTG + tg
                step = 1 if fine else 2
                for name in ("q", "k", "v"):
                    t = xtp.tile([P, XC, CW], BF16, tag=f"x{name}")
                    if fine and name == "q":
                        # halve the very first chunk so matmul 0 starts sooner
                        hw = CW // 2
                        nc.sync.dma_start(
                            t[:, 0, 0:hw], xdram[name][g, :, 0, 0:hw]
                        )
                        nc.sync.dma_start(
                            t[:, 0, hw:], xdram[name][g, :, 0, hw:]
                        )
                        first = 1
                    else:
                        first = 0
                    for xc in range(first, XC, step):
                        nc.sync.dma_start(
                            t[:, xc : xc + step, :],
                            xdram[name][g, :, xc : xc + step, :],
                        )
                    tiles[name] = t
                return tiles

            def emit_proj(b, name, tg, xtile):
                """Q/K projection of one 512-token group -> qt/kt columns."""
                ps = pp.tile([CH, CW], F32, tag="pp")
                for xc in range(XC):
                    nc.tensor.matmul(
                        ps[:],
                        lhsT=w_sb[name][:, xc, :],
                        rhs=xtile[:, xc, :],
                        start=(xc == 0),
                        stop=(xc == XC - 1),
                    )
                dst = qt if name == "q" else kt
                nc.vector.tensor_add(
                    dst[b][:, tg * CW : (tg + 1) * CW],
                    ps[:],
                    b_sb[name][:, 0:1].to_broadcast((CH, CW)),
                )

            def emit_proj_v(b, tg, xtile, vt_sb):
                """V projection of one token group in [chan, tok] layout."""
                ps = pp.tile([CH, CW], F32, tag="pp")
                for xc in range(XC):
                    nc.tensor.matmul(
                        ps[:],
                        lhsT=w_sb["v"][:, xc, :],
                        rhs=xtile[:, xc, :],
                        start=(xc == 0),
                        stop=(xc == XC - 1),
                    )
                nc.vector.tensor_copy(vt_sb[:], ps[:])

            def vt_unit(b, tg, tb, vt_sb):
                """Transpose one 128-token block of V into vaug [tok, chan]."""
                tp_ = pp.tile([P, HPC, DK], BF16, tag="pp")
                nc.tensor.transpose(
                    tp_[:], vt_sb[:, tb * P : (tb + 1) * P], ident[:]
                )
                kb = tg * KPG + tb
                nc.vector.tensor_copy(vaug[b][:, kb, :, 0:DK], tp_[:])

            def oproj_piece(tcol, yt, mo, last=False):
                op_ps = pp.tile([P, CW], F32, tag="pp")
                nc.tensor.matmul(
                    op_ps[:],
                    lhsT=wo_sb[:, mo * P : (mo + 1) * P],
                    rhs=yt[:],
                    start=True,
                    stop=True,
                )
                ob = obp.tile([P, CW], BF16, tag="ob")
                # evac split 2:6 ACT:DVE (the last column alternates evenly
                # and splits DMAs to shorten the kernel tail)
                on_act = mo % 2 == 0 if last else mo % 4 == 0
                if on_act:
                    nc.scalar.copy(ob[:], op_ps[:])
                else:
                    nc.vector.tensor_copy(ob[:], op_ps[:])
                if last:
                    # split across the SP and Pool DGE queues for a short tail
                    h = CW // 2
                    nc.sync.dma_start(out[mo, tcol][:, 0:h], ob[:, 0:h])
                    nc.gpsimd.dma_start(out[mo, tcol][:, h:], ob[:, h:])
                else:
                    nc.sync.dma_start(out[mo, tcol], ob[:])

            def attention_col(b, j, tg, vt_units, nxt_units, lazy, pace=450):
                """One 512-wide q column; drains filler units between blocks."""
                blocks = plan[j]
                q0 = j * CW
                yt = ytp.tile([CH, CW], BF16, tag="yt")
                if not blocks:
                    while vt_units:
                        _, fn = vt_units.popleft()
                        fn()
                    nc.gpsimd.memset(yt[:], 0.0)
                    return yt
                ops = [
                    opsp.tile([P, CW], F32, tag="op", name=f"op{hl}")
                    for hl in range(HPC)
                ]
                nblk = len(blocks)

                def emit_av(i, bk, qa, a2):
                    for hl in range(HPC):
                        nc.tensor.matmul(
                            ops[hl][:, qa:],
                            lhsT=vaug[b][:, bk, hl, :],
                            rhs=a2[:, hl, qa:],
                            start=(i == 0),
                            stop=(i == nblk - 1),
                            skip_group_check=True,
                        )

                pend = None
                debt = 0
                for i, (bk, qa, mixed) in enumerate(blocks):
                    k0 = bk * P
                    s2 = s2p.tile([P, HPC, CW], F32, tag="s2")
                    for hl in range(HPC):
                        hs = slice(hl * DK, (hl + 1) * DK)
                        nc.tensor.matmul(
                            s2[:, hl, qa:],
                            lhsT=kt[b][hs, k0 : k0 + P],
                            rhs=qt[b][hs, q0 + qa : q0 + CW],
                            start=True,
                            stop=True,
                            skip_group_check=True,
                        )
                    a2 = a2p.tile([P, HPC, CW], BF16, tag="a2")
                    nc.scalar.activation(
                        a2[:, :, qa:],
                        s2[:, :, qa:],
                        mybir.ActivationFunctionType.Exp,
                        scale=0.125,
                    )
                    if mixed is not None:
                        off, a_, w_ = mixed
                        nc.gpsimd.tensor_tensor(
                            a2[:, :, a_ : a_ + w_],
                            a2[:, :, a_ : a_ + w_],
                            mask_sb[:, None, off : off + w_].to_broadcast(
                                (P, HPC, w_)
                            ),
                            mybir.AluOpType.mult,
                        )
                    if pend is not None:
                        # this tg's V^T blocks must land before its AVs
                        if pend[1] >= tg * KPG:
                            while vt_units:
                                _, fn = vt_units.popleft()
                                fn()
                        emit_av(*pend)
                    pend = (i, bk, qa, a2)
                    # filler: keep the PE fed while the scalar engine exps
                    debt += pace
                    while debt > 0 and (vt_units or nxt_units or lazy):
                        src = (
                            vt_units
                            if vt_units
                            else (nxt_units if nxt_units else lazy)
                        )
                        ns, fn = src.popleft()
                        fn()
                        debt -= ns
                if pend[1] >= tg * KPG:
                    while vt_units:
                        _, fn = vt_units.popleft()
                        fn()
                emit_av(*pend)
                # normalize: rows 64..127 of ops are the sums, pre-broadcast.
                # DVE reads at most one PSUM operand per op, so reciprocal the
                # sums into SBUF, then multiply against the PSUM values.
                for hl in range(HPC):
                    # reciprocal_approx_fast misreads PSUM on HW — bounce the
                    # sums through SBUF (on ACT, to shorten the DVE chain)
                    sums = nrm.tile([DK, CW], F32, tag="sums", name=f"sums{hl}")
                    nc.scalar.copy(sums[:], ops[hl][DK:P, :])
                    rec = nrm.tile([DK, CW], F32, tag="rec", name=f"rec{hl}")
                    nc.vector.reciprocal_approx_fast(out=rec[:], in_=sums[:])
                    nc.vector.tensor_tensor(
                        yt[hl * DK : (hl + 1) * DK, :],
                        ops[hl][0:DK, :],
                        rec[:],
                        mybir.AluOpType.mult,
                    )
                return yt

            # --- main schedule ---------------------------------------------
            seq = [(b, tg) for b in range(B) for tg in range(NTG)]
            cur_x = load_x(*seq[0], fine=True)
            lazy = deque()
            for si, (b, tg) in enumerate(seq):
                nxt_x = load_x(*seq[si + 1]) if si + 1 < len(seq) else None
                emit_proj(b, "q", tg, cur_x["q"])
                emit_proj(b, "k", tg, cur_x["k"])
                vt_sb = vtp.tile([CH, CW], BF16, tag="vt")
                emit_proj_v(b, tg, cur_x["v"], vt_sb)
                vt_units = deque(
                    (450, partial(vt_unit, b, tg, tb, vt_sb))
                    for tb in range(KPG)
                )
                last = si == len(seq) - 1
                yt = attention_col(
                    b, tg, tg, vt_units, deque(), lazy,
                    pace=150 if last else 450,
                )
                while vt_units:
                    _, fn = vt_units.popleft()
                    fn()
                tcol = b * NJ + tg
                lazy.extend(
                    (450, partial(oproj_piece, tcol, yt, mo, last))
                    for mo in range(MO)
                )
                cur_x = nxt_x
            while lazy:
                _, fn = lazy.popleft()
                fn()
    nc.compile()
    return nc


def _get_module(plan, pat_w):
    key = (plan, pat_w)
    if key not in _BUILD_CACHE:
        _BUILD_CACHE[key] = _build(plan, pat_w)
    return _BUILD_CACHE[key]


def _prep_inputs(query, key, value, mask, W_q, b_q, W_k, b_k, W_v, b_v, W_o, b_o):
    def xt_of(x):
        x2 = np.asarray(x, np.float32).reshape(TOK, D)
        xt = x2.T.astype(NPBF16)  # (D, TOK)
        xt = xt.reshape(XC, P, B * NTG, CW).transpose(2, 1, 0, 3)
        return np.ascontiguousarray(xt)  # (NTT, P, XC, CW)

    xq, xk, xv = xt_of(query), xt_of(key), xt_of(value)
    plan, pat_arr = _analyze_mask(mask)
    mpat = np.ascontiguousarray(pat_arr).astype(NPBF16)

    W_q = np.asarray(W_q, np.float32)
    W_k = np.asarray(W_k, np.float32)
    W_v = np.asarray(W_v, np.float32)
    W_o = np.asarray(W_o, np.float32)

    in_maps = []
    for c in range(N_CORES):
        cs = slice(c * CH, (c + 1) * CH)
        in_maps.append(
            {
                "xq": xq,
                "xk": xk,
                "xv": xv,
                "wq": np.ascontiguousarray(W_q[cs, :].T).astype(NPBF16),
                "wk": np.ascontiguousarray(W_k[cs, :].T).astype(NPBF16),
                "wv": np.ascontiguousarray(W_v[cs, :].T).astype(NPBF16),
                "wo": np.ascontiguousarray(W_o[:, cs].T).astype(NPBF16),
                "bq": np.asarray(b_q, np.float32)[cs].reshape(CH, 1).copy(),
                "bk": np.asarray(b_k, np.float32)[cs].reshape(CH, 1).copy(),
                "mpat": mpat,
            }
        )
    return plan, mpat.shape[1], in_maps


def run(inputs, trace=False, trace_cores=None):
    """Build (cached), run on 8 cores, return (final_output, BassKernelResults)."""
    plan, pat_w, in_maps = _prep_inputs(**inputs)
    nc = _get_module(plan, pat_w)
    res = bass_utils.run_bass_kernel_spmd(
        nc,
        in_maps,
        core_ids=list(range(N_CORES)),
        trace=trace,
        trace_cores=trace_cores,
    )
    acc = np.zeros((MO, B * NJ, P, CW), np.float32)
    for c in range(N_CORES):
        acc += res.results[c]["out"].astype(np.float32)
    acc = acc.transpose(0, 2, 1, 3).reshape(D, TOK)
    # v-bias contributes W_o @ b_v to every token; fold it into the out bias
    bo_eff = np.asarray(inputs["b_o"], np.float32) + np.asarray(
        inputs["W_o"], np.float32
    ) @ np.asarray(inputs["b_v"], np.float32)
    final = acc.T + bo_eff[None, :]
    return final.reshape(B, S, D), res


def kernel(**inputs):
    return run(inputs, trace=False)[0]


# revision 35
# speedup vs baseline: 1.0673x; 1.0165x over previous
"""Multi-head attention (B=2, S=2048, D=1024, H=16 heads, causal) on 8 trn2 cores.

Sharding: heads across cores (2 heads = 128 channels per core).
  - W_q/W_k/W_v column-sharded: each core projects all tokens to its 128 channels.
  - Attention per (batch, head) fully local to a core.
  - W_o row-sharded: each core computes a partial output projection; partials
    are summed on the host (the unshard step), then b_o (+ W_o @ b_v) is added.

Device layout: Q/K transposed (channels on partitions, tokens on free).
  - Scores computed as S^T blocks [128 k-tok, 512 q-tok] so exp is elementwise.
  - V^T produced directly by the projection (lhsT=x chunk, rhs=W_v chunk), no
    PE transposes. Each (b, head) V block carries 64 all-ones columns, so the
    AV matmul emits the softmax sums pre-broadcast across 64 partitions; the
    normalize is then a single tensor-tensor divide per head.
  - Causal structure: scores/exp/AV restricted to the valid q-range per
    k-block; the 127-wide diagonal triangle is multiplied in on GpSimd.
  - Projections / V^T pieces / output-projection pieces are emitted as filler
    between attention blocks so the PE never idles (keeps the 2.4 GHz p-state)
    while the scalar engine runs the exp stream.

All matmuls run in bf16 (inputs cast on host) with fp32 PSUM accumulation;
the partial output is returned bf16 and reduced in fp32 on the host.
"""

import sys
from collections import deque
from functools import partial

import numpy as np

try:
    import concourse.bass as bass  # noqa: F401
except ImportError:  # pragma: no cover
    sys.path.insert(0, "/opt/trn_rl_repo")

import ml_dtypes

import concourse.mybir as mybir
import concourse.tile as tile
from concourse import bacc, bass_utils
from concourse.masks import make_identity

P = 128
B, S, D = 2, 2048, 1024
H, DK = 16, 64
N_CORES = 8
HPC = H // N_CORES  # heads per core = 2
CH = HPC * DK  # channels per core = 128
TOK = B * S  # 4096
NKB = S // P  # k-blocks per batch = 16
CW = 512  # q column width
NJ = S // CW  # q columns per batch = 4
NTG = S // CW  # 512-token projection groups per batch = 4
KPG = CW // P  # k-blocks per token group = 4
XC = D // P  # x-dim chunks = 8
MO = D // P  # output-channel chunks = 8

BF16 = mybir.dt.bfloat16
F32 = mybir.dt.float32
NPBF16 = ml_dtypes.bfloat16

_BUILD_CACHE = {}


def _analyze_mask(mask):
    """Block plan from the (1,1,S,S) boolean mask (shared across batch/head).

    plan[j] = tuple of (bk, qa, mixed) for each k-block with any valid entry:
      qa    = first local q with any valid k; scores/exp/AV cover [qa, CW).
      mixed = None or (pat_off, a, w): a2[:, :, a:a+w] *= pattern columns.
    Patterns are deduplicated and concatenated into pats (P, W_total) in
    [k, q] layout.
    """
    m = np.asarray(mask).reshape(S, S).astype(bool)  # m[q, k]
    pat_index = {}
    pat_list = []
    plan = []
    for j in range(NJ):
        q0 = j * CW
        blocks = []
        first = True
        for bk in range(NKB):
            sub = m[q0 : q0 + CW, bk * P : (bk + 1) * P]  # (CW q, P k)
            anyv = sub.any(axis=1)
            if not anyv.any():
                continue
            qa = int(np.argmax(anyv))
            if first:
                # the first block initializes the whole PSUM accumulator
                qa = 0
                first = False
            validall = sub.all(axis=1)
            nfv = ~validall
            nfv[:qa] = False
            mixed = None
            if nfv.any():
                idx = np.where(nfv)[0]
                a_, b_ = int(idx[0]), int(idx[-1]) + 1
                patt = np.ascontiguousarray(sub[a_:b_, :].T).astype(np.float32)
                key = (patt.shape[1], patt.tobytes())
                if key not in pat_index:
                    pat_index[key] = len(pat_list)
                    pat_list.append(patt)
                mixed = (pat_index[key], a_, b_ - a_)
            blocks.append((bk, qa, mixed))
        plan.append(tuple(blocks))
    offs = [0]
    for p_ in pat_list:
        offs.append(offs[-1] + p_.shape[1])
    plan2 = []
    for col in plan:
        col2 = []
        for bk, qa, mixed in col:
            if mixed is not None:
                pid, a_, w_ = mixed
                mixed = (offs[pid], a_, w_)
            col2.append((bk, qa, mixed))
        plan2.append(tuple(col2))
    if pat_list:
        pat_arr = np.concatenate(pat_list, axis=1)  # (P, W_total)
    else:
        pat_arr = np.ones((P, 1), np.float32)
    return tuple(plan2), pat_arr


def _build(plan, pat_w):
    nc = bacc.Bacc(
        "TRN2",
        target_bir_lowering=False,
        debug=False,
        enable_asserts=True,
        num_devices=N_CORES,
    )
    NTT = B * NTG
    xq = nc.dram_tensor("xq", [NTT, P, XC, CW], BF16, kind="ExternalInput").ap()
    xk = nc.dram_tensor("xk", [NTT, P, XC, CW], BF16, kind="ExternalInput").ap()
    xv = nc.dram_tensor("xv", [NTT, P, XC, CW], BF16, kind="ExternalInput").ap()
    wq = nc.dram_tensor("wq", [D, CH], BF16, kind="ExternalInput").ap()
    wk = nc.dram_tensor("wk", [D, CH], BF16, kind="ExternalInput").ap()
    wv = nc.dram_tensor("wv", [D, CH], BF16, kind="ExternalInput").ap()
    wo = nc.dram_tensor("wo", [CH, D], BF16, kind="ExternalInput").ap()
    bq = nc.dram_tensor("bq", [CH, 1], F32, kind="ExternalInput").ap()
    bk_ = nc.dram_tensor("bk", [CH, 1], F32, kind="ExternalInput").ap()
    mpat = nc.dram_tensor("mpat", [P, pat_w], BF16, kind="ExternalInput").ap()
    out = nc.dram_tensor(
        "out", [MO, B * NJ, P, CW], BF16, kind="ExternalOutput"
    ).ap()

    xdram = {"q": xq, "k": xk, "v": xv}

    with tile.TileContext(nc) as tc:
        with (
            tc.tile_pool(name="const", bufs=1) as const,
            tc.tile_pool(name="persist", bufs=1) as persist,
            tc.tile_pool(name="xt", bufs=2) as xtp,
            tc.tile_pool(name="vt", bufs=3) as vtp,
            tc.tile_pool(name="a2", bufs=5) as a2p,
            tc.tile_pool(name="nrm", bufs=4) as nrm,
            tc.tile_pool(name="yt", bufs=4) as ytp,
            tc.tile_pool(name="ob", bufs=16) as obp,
            tc.tile_pool(name="pp", bufs=2, space="PSUM") as pp,
            tc.tile_pool(name="s2", bufs=2, space="PSUM") as s2p,
            tc.tile_pool(name="op", bufs=2, space="PSUM") as opsp,
        ):
            # --- constants: loaded via the Activation HWDGE queue so the SP
            # queue is dedicated to x prefetch ------------------------------
            w_sb = {}
            b_sb = {}
            for name, wdram, bdram in (
                ("q", wq, bq),
                ("k", wk, bk_),
                ("v", wv, None),
            ):
                w_sb[name] = const.tile(
                    [P, XC, CH], BF16, tag=f"w{name}", name=f"w{name}"
                )
                wview = wdram.rearrange("(o p) c -> p o c", p=P)
                # wq chunk 0 gates the first matmul — load it alone
                bounds = [0, 1, 2, 4, 6, 8] if name == "q" else [0, 2, 4, 6, 8]
                for lo, hi in zip(bounds, bounds[1:]):
                    nc.scalar.dma_start(
                        w_sb[name][:, lo:hi, :], wview[:, lo:hi, :]
                    )
                if bdram is not None:
                    b_sb[name] = const.tile(
                        [CH, 1], F32, tag=f"b{name}", name=f"b{name}"
                    )
                    nc.scalar.dma_start(b_sb[name][:], bdram)
            wo_sb = const.tile([CH, D], BF16, tag="wo")
            for h in range(0, D, CW):
                nc.scalar.dma_start(wo_sb[:, h : h + CW], wo[:, h : h + CW])
            mask_sb = const.tile([P, pat_w], BF16, tag="mpat")
            nc.scalar.dma_start(mask_sb[:], mpat)
            ident = const.tile([P, P], BF16, tag="ident")
            make_identity(nc, ident)

            # persistent Q^T/K^T [chan, tok] and V^T-augmented per batch
            qt, kt = {}, {}
            for b in range(B):
                qt[b] = persist.tile([CH, S], BF16, tag=f"qt{b}", name=f"qt{b}")
                kt[b] = persist.tile([CH, S], BF16, tag=f"kt{b}", name=f"kt{b}")
            # vaug[b]: [k-tok, NKB, head, 64 V cols + 64 ones cols]; the ones
            # make the AV matmul emit softmax sums broadcast on rows 64..127
            vaug = {}
            for b in range(B):
                vaug[b] = persist.tile(
                    [P, NKB, HPC, P], BF16, tag=f"vaug{b}", name=f"vaug{b}"
                )
                # contiguous fill; the V columns are overwritten by the V^T
                # evacs, leaving the ones columns that produce the sums rows
                nc.gpsimd.memset(vaug[b][:], 1.0)

            def load_x(b, tg, fine=False):
                """Prefetch x tiles; fine=True splits into 128KB chunks for a
                fast first matmul, otherwise 256KB halves the SP setup cost."""
                tiles = {}
                g = b * NTG + tg
                step = 1 if fine else 2
                for name in ("q", "k", "v"):
                    t = xtp.tile([P, XC, CW], BF16, tag=f"x{name}")
                    if fine and name == "q":
                        # halve the very first chunk so matmul 0 starts sooner
                        hw = CW // 2
                        nc.sync.dma_start(
                            t[:, 0, 0:hw], xdram[name][g, :, 0, 0:hw]
                        )
                        nc.sync.dma_start(
                            t[:, 0, hw:], xdram[name][g, :, 0, hw:]
                        )
                        first = 1
                    else:
                        first = 0
                    for xc in range(first, XC, step):
                        nc.sync.dma_start(
                            t[:, xc : xc + step, :],
                            xdram[name][g, :, xc : xc + step, :],
                        )
                    tiles[name] = t
                return tiles

            def emit_proj(b, name, tg, xtile):
                """Q/K projection of one 512-token group -> qt/kt columns."""
                ps = pp.tile([CH, CW], F32, tag="pp")
                for xc in range(XC):
                    nc.tensor.matmul(
                        ps[:],
                        lhsT=w_sb[name][:, xc, :],
                        rhs=xtile[:, xc, :],
                        start=(xc == 0),
                        stop=(xc == XC - 1),
                    )
                dst = qt if name == "q" else kt
                nc.vector.tensor_add(
                    dst[b][:, tg * CW : (tg + 1) * CW],
                    ps[:],
                    b_sb[name][:, 0:1].to_broadcast((CH, CW)),
                )

            def emit_proj_v(b, tg, xtile, vt_sb):
                """V projection of one token group in [chan, tok] layout."""
                ps = pp.tile([CH, CW], F32, tag="pp")
                for xc in range(XC):
                    nc.tensor.matmul(
                        ps[:],
                        lhsT=w_sb["v"][:, xc, :],
                        rhs=xtile[:, xc, :],
                        start=(xc == 0),
                        stop=(xc == XC - 1),
                    )
                nc.vector.tensor_copy(vt_sb[:], ps[:])

            def vt_unit(b, tg, tb, vt_sb):
                """Transpose one 128-token block of V into vaug [tok, chan]."""
                tp_ = pp.tile([P, HPC, DK], BF16, tag="pp")
                nc.tensor.transpose(
                    tp_[:], vt_sb[:, tb * P : (tb + 1) * P], ident[:]
                )
                kb = tg * KPG + tb
                nc.vector.tensor_copy(vaug[b][:, kb, :, 0:DK], tp_[:])

            def oproj_piece(tcol, yt, mo, last=False):
                op_ps = pp.tile([P, CW], F32, tag="pp")
                nc.tensor.matmul(
                    op_ps[:],
                    lhsT=wo_sb[:, mo * P : (mo + 1) * P],
                    rhs=yt[:],
                    start=True,
                    stop=True,
                )
                ob = obp.tile([P, CW], BF16, tag="ob")
                # evac on DVE so the scalar engine's exp stream stays clean;
                # the last column alternates engines to shorten the tail
                on_act = mo % 2 == 0 if last else False
                if on_act:
                    nc.scalar.copy(ob[:], op_ps[:])
                else:
                    nc.vector.tensor_copy(ob[:], op_ps[:])
                if last:
                    # split across the SP and Pool DGE queues for a short tail
                    h = CW // 2
                    nc.sync.dma_start(out[mo, tcol][:, 0:h], ob[:, 0:h])
                    nc.gpsimd.dma_start(out[mo, tcol][:, h:], ob[:, h:])
                else:
                    nc.sync.dma_start(out[mo, tcol], ob[:])

            def attention_col(b, j, tg, vt_units, nxt_units, lazy, pace=450):
                """One 512-wide q column; drains filler units between blocks."""
                blocks = plan[j]
                q0 = j * CW
                yt = ytp.tile([CH, CW], BF16, tag="yt")
                if not blocks:
                    while vt_units:
                        _, fn = vt_units.popleft()
                        fn()
                    nc.gpsimd.memset(yt[:], 0.0)
                    return yt
                ops = [
                    opsp.tile([P, CW], F32, tag="op", name=f"op{hl}")
                    for hl in range(HPC)
                ]
                nblk = len(blocks)

                def emit_av(i, bk, qa, a2):
                    for hl in range(HPC):
                        nc.tensor.matmul(
                            ops[hl][:, qa:],
                            lhsT=vaug[b][:, bk, hl, :],
                            rhs=a2[:, hl, qa:],
                            start=(i == 0),
                            stop=(i == nblk - 1),
                            skip_group_check=True,
                        )

                pend = None
                debt = 0
                for i, (bk, qa, mixed) in enumerate(blocks):
                    k0 = bk * P
                    s2 = s2p.tile([P, HPC, CW], F32, tag="s2")
                    for hl in range(HPC):
                        hs = slice(hl * DK, (hl + 1) * DK)
                        nc.tensor.matmul(
                            s2[:, hl, qa:],
                            lhsT=kt[b][hs, k0 : k0 + P],
                            rhs=qt[b][hs, q0 + qa : q0 + CW],
                            start=True,
                            stop=True,
                            skip_group_check=True,
                        )
                    a2 = a2p.tile([P, HPC, CW], BF16, tag="a2")
                    nc.scalar.activation(
                        a2[:, :, qa:],
                        s2[:, :, qa:],
                        mybir.ActivationFunctionType.Exp,
                        scale=0.125,
                    )
                    if mixed is not None:
                        off, a_, w_ = mixed
                        nc.gpsimd.tensor_tensor(
                            a2[:, :, a_ : a_ + w_],
                            a2[:, :, a_ : a_ + w_],
                            mask_sb[:, None, off : off + w_].to_broadcast(
                                (P, HPC, w_)
                            ),
                            mybir.AluOpType.mult,
                        )
                    if pend is not None:
                        # this tg's V^T blocks must land before its AVs
                        if pend[1] >= tg * KPG:
                            while vt_units:
                                _, fn = vt_units.popleft()
                                fn()
                        emit_av(*pend)
                    pend = (i, bk, qa, a2)
                    # filler: keep the PE fed while the scalar engine exps
                    debt += pace
                    while debt > 0 and (vt_units or nxt_units or lazy):
                        src = (
                            vt_units
                            if vt_units
                            else (nxt_units if nxt_units else lazy)
                        )
                        ns, fn = src.popleft()
                        fn()
                        debt -= ns
                if pend[1] >= tg * KPG:
                    while vt_units:
                        _, fn = vt_units.popleft()
                        fn()
                emit_av(*pend)
                # normalize: rows 64..127 of ops are the sums, pre-broadcast.
                # DVE reads at most one PSUM operand per op, so reciprocal the
                # sums into SBUF, then multiply against the PSUM values.
                for hl in range(HPC):
                    # reciprocal_approx_fast misreads PSUM on HW — bounce the
                    # sums through SBUF (on ACT, to shorten the DVE chain)
                    sums = nrm.tile([DK, CW], F32, tag="sums", name=f"sums{hl}")
                    nc.scalar.copy(sums[:], ops[hl][DK:P, :])
                    rec = nrm.tile([DK, CW], F32, tag="rec", name=f"rec{hl}")
                    nc.vector.reciprocal_approx_fast(out=rec[:], in_=sums[:])
                    nc.vector.tensor_tensor(
                        yt[hl * DK : (hl + 1) * DK, :],
                        ops[hl][0:DK, :],
                        rec[:],
                        mybir.AluOpType.mult,
                    )
                return yt

            # --- main schedule ---------------------------------------------
            seq = [(b, tg) for b in range(B) for tg in range(NTG)]
            cur_x = load_x(*seq[0], fine=True)
            lazy = deque()
            for si, (b, tg) in enumerate(seq):
                nxt_x = load_x(*seq[si + 1]) if si + 1 < len(seq) else None
                emit_proj(b, "q", tg, cur_x["q"])
                emit_proj(b, "k", tg, cur_x["k"])
                vt_sb = vtp.tile([CH, CW], BF16, tag="vt")
                emit_proj_v(b, tg, cur_x["v"], vt_sb)
                vt_units = deque(
                    (450, partial(vt_unit, b, tg, tb, vt_sb))
                    for tb in range(KPG)
                )
                last = si == len(seq) - 1
                yt = attention_col(
                    b, tg, tg, vt_units, deque(), lazy,
                    pace=150 if last else 450,
                )
                while vt_units:
                    _, fn = vt_units.popleft()
                    fn()
                tcol = b * NJ + tg
                lazy.extend(
                    (450, partial(oproj_piece, tcol, yt, mo, last))
                    for mo in range(MO)
                )
                cur_x = nxt_x
            while lazy:
                _, fn = lazy.popleft()
                fn()
    nc.compile()
    return nc


def _get_module(plan, pat_w):
    key = (plan, pat_w)
    if key not in _BUILD_CACHE:
        _BUILD_CACHE[key] = _build(plan, pat_w)
    return _BUILD_CACHE[key]


def _prep_inputs(query, key, value, mask, W_q, b_q, W_k, b_k, W_v, b_v, W_o, b_o):
    def xt_of(x):
        x2 = np.asarray(x, np.float32).reshape(TOK, D)
        xt = x2.T.astype(NPBF16)  # (D, TOK)
        xt = xt.reshape(XC, P, B * NTG, CW).transpose(2, 1, 0, 3)
        return np.ascontiguousarray(xt)  # (NTT, P, XC, CW)

    xq, xk, xv = xt_of(query), xt_of(key), xt_of(value)
    plan, pat_arr = _analyze_mask(mask)
    mpat = np.ascontiguousarray(pat_arr).astype(NPBF16)

    W_q = np.asarray(W_q, np.float32)
    W_k = np.asarray(W_k, np.float32)
    W_v = np.asarray(W_v, np.float32)
    W_o = np.asarray(W_o, np.float32)

    in_maps = []
    for c in range(N_CORES):
        cs = slice(c * CH, (c + 1) * CH)
        in_maps.append(
            {
                "xq": xq,
                "xk": xk,
                "xv": xv,
                "wq": np.ascontiguousarray(W_q[cs, :].T).astype(NPBF16),
                "wk": np.ascontiguousarray(W_k[cs, :].T).astype(NPBF16),
                "wv": np.ascontiguousarray(W_v[cs, :].T).astype(NPBF16),
                "wo": np.ascontiguousarray(W_o[:, cs].T).astype(NPBF16),
                "bq": np.asarray(b_q, np.float32)[cs].reshape(CH, 1).copy(),
                "bk": np.asarray(b_k, np.float32)[cs].reshape(CH, 1).copy(),
                "mpat": mpat,
            }
        )
    return plan, mpat.shape[1], in_maps


def run(inputs, trace=False, trace_cores=None):
    """Build (cached), run on 8 cores, return (final_output, BassKernelResults)."""
    plan, pat_w, in_maps = _prep_inputs(**inputs)
    nc = _get_module(plan, pat_w)
    res = bass_utils.run_bass_kernel_spmd(
        nc,
        in_maps,
        core_ids=list(range(N_CORES)),
        trace=trace,
        trace_cores=trace_cores,
    )
    acc = np.zeros((MO, B * NJ, P, CW), np.float32)
    for c in range(N_CORES):
        acc += res.results[c]["out"].astype(np.float32)
    acc = acc.transpose(0, 2, 1, 3).reshape(D, TOK)
    # v-bias contributes W_o @ b_v to every token; fold it into the out bias
    bo_eff = np.asarray(inputs["b_o"], np.float32) + np.asarray(
        inputs["W_o"], np.float32
    ) @ np.asarray(inputs["b_v"], np.float32)
    final = acc.T + bo_eff[None, :]
    return final.reshape(B, S, D), res


def kernel(**inputs):
    return run(inputs, trace=False)[0]
